# revision 76
# baseline (speedup 1.0000x reference)
"""Distributed Trainium2 Bass kernel for a 3-layer GCN (ArithmeticCircuitGNN).

Self-contained: takes full inputs, shards nodes across 8 NeuronCores,
runs the compiled Bass graph via run_bass_kernel_spmd, returns full output.

Math per GCN layer (reference: PyG GCNConv with self-loops):
    out = Dinv (A + I) Dinv (h) W + b        with Dinv = diag(deg^-1/2)
We fold the two Dinv factors into per-node scalings:
    hs   = dinv * h                 (source-side, before gather)
    agg  = (A + I) hs               (gather + one-hot matmul scatter-add)
    out  = (dinv * agg) W + b       (dst-side scale, then weight matmul)

Perf structure (2.39 ms vs 5.64 ms baseline; bottleneck is GPSIMD/Q7
SWDGE descriptor generation at ~1.7 us per 640-index gather call):
  - layer-1 gather tables are host-precomputed (dinv*x in bf16) and fed as
    input params: no conv-pre phase, no layer-1 AllGather, ~25 us startup.
  - gather calls are aligned to (dst-block, sub-table) groups; each core's
    trailing pad lanes carry idx=-1 (dropped by the ucode's trailing-
    negative trim) with num_idxs_reg loaded per-core from SBUF via
    batched reg_load, so ring reservation == emitted descriptors. This
    cuts ~20% of the random 256B HBM reads and keeps calls single-packet
    (<= 64 descriptors per engine).
  - stale-lane safety: trimmed lanes leave old SBUF bytes in the gather
    tile (masked by dstloc=200 -> S=0), so every pool buffer is memset to
    zero once at startup - 0 * NaN from uninitialized SBUF would
    otherwise poison the psum accumulation.
  - tile stream is dst-half-major then sub-major: per-block epilogues run
    inside each half's last sub pass, and each AllGather fires mid-layer
    with ~half a layer of gather stream to hide behind; table/shard
    buffers ping-pong between layers so an early AllGather never
    overwrites a table the current layer still reads.
  - the self-loop term enters each block's first psum chain as an
    identity matmul; the dst-side dinv rides the transpose matmul as a
    host-built diagonal (dropped entirely for layer 3: the final
    LayerNorm is row-scale-invariant); the next layer's src-side dinv is
    folded into the ReLU scale (h1 residual stored pre-scaled) and into
    the LayerNorm's rstd; LN computes var = E[h^2]-mu^2 and emits
    (h-mu)*rstd as one dual-op tensor_scalar, straight to bf16 hs_pre.
"""

import contextlib
import ctypes
import os
import sys
import types

import numpy as np
import ml_dtypes

import concourse.bass as bass
import concourse.mybir as mybir
import concourse.tile as tile
from concourse import bacc
from concourse.bass_utils import run_bass_kernel_spmd

# ---------------- problem constants (hardcoded per spec) ----------------
N = 100000
E = 1600000
D = 128
P = 128
NCORE = 8
BPC = 98                 # dst blocks of 128 nodes per core
SH = BPC * P             # 12544 nodes per core shard
NPAD = NCORE * SH        # 100352 padded node count
NSUB = 4                 # sub-tables (int16 index reach)
SUBROWS = NPAD // NSUB   # 25088 rows per sub-table
NIMAX_TILES = 5          # max tiles per dma_gather call (= largest group)
MERGE_CALLS = False      # merged calls need multi-packet descriptors, which
                         # drain slower than the saved per-call fixed cost
PADLOC = 200.0           # dstloc value for padding lanes (> 127)
GP_BUFS = 36             # gather-pool depth
NOTRIM_CALLS = 0         # gather-pool buffers are memset once at startup, so
                         # every call can trim its pad lanes (idx=-1); trimmed
                         # lanes then read zeros (finite), never stale NaN bits
LN_EPS = 1e-5

BF16 = mybir.dt.bfloat16
F32 = mybir.dt.float32
I16 = mybir.dt.int16

EXEC_TIME_NS = None      # set by kernel() when profiling is enabled


# ---------------- axon NTFF profile hook (optional) ----------------
def _install_profile_hook():
    so_path = "/opt/axon/libaxon_pjrt.so"
    if "antenv.axon_hooks" in sys.modules:
        return True
    try:
        lib = ctypes.CDLL(so_path)
        if not hasattr(lib, "axon_start_nrt_profile"):
            return False
        lib.axon_start_nrt_profile.argtypes = [ctypes.POINTER(ctypes.c_int64), ctypes.c_size_t]
        lib.axon_start_nrt_profile.restype = ctypes.c_int64
        lib.axon_stop_nrt_profile.argtypes = [ctypes.c_char_p]
        lib.axon_stop_nrt_profile.restype = ctypes.c_int64

        @contextlib.contextmanager
        def _hook(output_dir, device_ids):
            import jax
            jax.devices()
            if device_ids:
                ids = (ctypes.c_int64 * len(device_ids))(*device_ids)
                rc = lib.axon_start_nrt_profile(ids, len(device_ids))
            else:
                rc = lib.axon_start_nrt_profile(None, 0)
            if rc != 0:
                raise RuntimeError(f"axon_start_nrt_profile rc={rc}")
            try:
                yield
            finally:
                n = lib.axon_stop_nrt_profile(str(output_dir).encode())
                if n < 0:
                    raise RuntimeError(f"axon_stop_nrt_profile rc={n}")

        mod = types.ModuleType("antenv.axon_hooks")
        mod.get_axon_ntff_profile_hook = lambda: _hook
        mod.set_axon_ntff_profile_hook = lambda h: None
        sys.modules["antenv.axon_hooks"] = mod

        import concourse.bass_utils as bu
        bu.upload_artifacts = lambda tmpdir: f"local:{tmpdir}"
        return True
    except Exception:
        return False


# ---------------- host-side graph preprocessing ----------------
def _preprocess(edge_index):
    src = np.asarray(edge_index[0], dtype=np.int64)
    dst = np.asarray(edge_index[1], dtype=np.int64)

    deg = np.bincount(dst, minlength=NPAD).astype(np.float64) + 1.0
    dinv = (1.0 / np.sqrt(deg)).astype(np.float32)  # padding nodes -> 1.0

    # table row of node g: owner rank halves are concatenated into two
    # half-tables (A = first 6272 rows of every rank, B = second half).
    HSH = SH // 2
    r_own = src // SH
    off = src % SH
    half = off // HSH
    lrow = r_own * HSH + (off % HSH)          # row within half-table
    sub = half * 2 + lrow // SUBROWS          # 0..3
    srcloc_all = lrow % SUBROWS
    gblk = dst // P                           # global dst block 0..781
    key = gblk * NSUB + sub
    order = np.argsort(key, kind="stable")
    src_s, dst_s, key_s = src[order], dst[order], key[order]
    srcloc_s = srcloc_all[order]

    NKEY = NCORE * BPC * NSUB                 # 784*4 (incl. empty tail blocks)
    cnt = np.bincount(key_s, minlength=NKEY)
    # per (core, local block, sub) counts; blocks 782/783 are zero
    cnt_cbs = cnt.reshape(NCORE, BPC, NSUB)
    T_u = np.ceil(cnt_cbs / P).astype(np.int64).max(axis=0)  # [BPC, NSUB]

    # tile stream: for dst-block half: for s in 0..3: for bl in half.
    # Half-major ordering closes blocks 0-48 (shard A) at ~50% of the layer
    # so the next layer's AllGather A can fire mid-layer; sub-major within a
    # half keeps table_b unneeded until ~25% into the stream.
    HB = BPC // 2 + BPC % 2                   # 49 blocks in the first half
    blorder = [(s, bl) for half in (range(HB), range(HB, BPC))
               for s in range(NSUB) for bl in half]
    NTILES = int(T_u.sum())

    # column base of each (s, bl) group in the tile stream
    group_base = np.zeros((NSUB, BPC), dtype=np.int64)
    cur = 0
    for s, bl in blorder:
        group_base[s, bl] = cur
        cur += int(T_u[bl, s])
    assert cur == NTILES

    # gather calls: merge same-s stream-adjacent groups per call (they read
    # the same sub-table), amortizing the ~1.5us Q7 per-call fixed cost.
    # Only the call-trailing pads (the last group's tail) can be dropped by
    # the ucode's trailing-negative trim; earlier merged groups' pads stay
    # idx=0 and are fetched (cheap, the drain has slack).
    # calls: list of (s, groups, ntiles) with groups = [(bl, T), ...].
    glist = [(s, bl, int(T_u[bl, s])) for s, bl in blorder if T_u[bl, s] > 0]
    calls = []                                # (s, groups, ntiles, off_tiles)
    i = 0
    while i < len(glist):
        s, bl, T = glist[i]
        if T > NIMAX_TILES:                   # oversized group: chunk it
            done = 0
            while done < T:
                ch = min(NIMAX_TILES, T - done)
                calls.append((s, [(bl, ch)], ch, done))
                done += ch
            i += 1
            continue
        groups = [(bl, T)]
        nt = T
        while (MERGE_CALLS and i + 1 < len(glist) and glist[i + 1][0] == s
               and nt + glist[i + 1][2] <= NIMAX_TILES):
            i += 1
            groups.append((glist[i][1], glist[i][2]))
            nt += glist[i][2]
        calls.append((s, groups, nt, 0))
        i += 1

    # tile meta: (bl, s, first_of_group, last_of_group) in stream order
    tilemeta = []
    for s, bl in blorder:
        T = int(T_u[bl, s])
        for t in range(T):
            tilemeta.append((bl, s, t == 0, t == T - 1))

    # per-core edge placement
    starts = np.zeros(NKEY + 1, dtype=np.int64)
    starts[1:] = np.cumsum(cnt)
    rank = np.arange(len(src_s)) - np.repeat(starts[:-1], cnt)

    core_e = gblk[order] // BPC               # owning core of each (sorted) edge
    bl_e = gblk[order] % BPC
    sub_e = key_s % NSUB
    pos = group_base[sub_e, bl_e] * P + rank  # slot in the core's edge stream

    src_local = srcloc_s.astype(np.int16)
    dst_local = (dst_s - (core_e * SH + bl_e * P)).astype(np.float32)

    srcbuf = np.full((NCORE, NTILES * P), -1, dtype=np.int16)
    dstbuf = np.full((NCORE, NTILES * P), PADLOC, dtype=np.float32)
    for c in range(NCORE):
        m = core_e == c
        srcbuf[c, pos[m]] = src_local[m]
        dstbuf[c, pos[m]] = dst_local[m]

    # dstloc sbuf layout: [p, tile]
    dstloc = dstbuf.reshape(NCORE, NTILES, P).transpose(0, 2, 1)  # [c, 128, NTILES]

    # per-core per-call valid-index counts: the gather's num_idxs_reg must
    # equal the number of non-negative indices (the ucode's ring-space
    # reservation and trigger counts are reg-based while descriptor emission
    # is trimmed-data-based; they must agree or stale descriptors fire).
    # Within a call, all groups but the last count full T*128 lanes (their
    # pads are idx=0); the last group is trimmed to this core's edge count.
    ccnt = np.zeros((NCORE, len(calls)), dtype=np.uint32)
    tc0 = 0
    for ci, (s, groups, nt, off) in enumerate(calls):
        if ci < NOTRIM_CALLS:
            ccnt[:, ci] = nt * P
        else:
            bl_last, T_last = groups[-1]
            full = (nt - T_last) * P
            ccnt[:, ci] = full + np.clip(
                cnt_cbs[:, bl_last, s] - off * P, 0, T_last * P)
        tc0 += nt
    assert tc0 == NTILES
    # exact per-call num_idxs (shared immediate): the Q7 widen loop runs
    # ceil(num_idxs/16) iterations, so shave it to the max core's count
    nidx = ((ccnt.max(axis=0).astype(np.int64) + 15) // 16 * 16)

    # pad-lane idx values: -1 (trimmed) only in each call's LAST group;
    # earlier merged groups' pads become 0 (fetched, masked by dstloc=200).
    tc0 = 0
    for ci, (s, groups, nt, off) in enumerate(calls):
        bl_last, T_last = groups[-1]
        lo = tc0 * P
        hi_nontrim = (tc0 + nt - T_last) * P
        if ci < NOTRIM_CALLS:
            hi_nontrim = (tc0 + nt) * P
        seg = srcbuf[:, lo:hi_nontrim]
        seg[seg < 0] = 0
        tc0 += nt
    assert tc0 == NTILES

    # idx16 layout per call: element i -> [i%16, base + i//16], replicated x8.
    idxcols = NTILES * (P // 16)
    idxbuf = np.zeros((NCORE, 16, idxcols), dtype=np.int16)
    tc = 0
    colc = 0
    for ci, (s, groups, nt, off) in enumerate(calls):
        n = nt * P
        blk = srcbuf[:, tc * P:tc * P + n].reshape(NCORE, n // 16, 16)
        idxbuf[:, :, colc:colc + n // 16] = blk.transpose(0, 2, 1)
        tc += nt
        colc += n // 16
    assert tc == NTILES and colc == idxcols
    idx_arr = np.tile(idxbuf, (1, 8, 1))      # [c, 128, idxcols]

    meta = {
        "NTILES": NTILES,
        "IDXCOLS": idxcols,
        "calls": calls,
        "tilemeta": tilemeta,
        "nidx": nidx,
    }
    data = {
        "idx": idx_arr,
        "dstloc": dstloc.astype(ml_dtypes.bfloat16),
        "dinv": dinv,
        "ccnt": ccnt,
    }
    return meta, data


# ---------------- device graph ----------------
def _ap3_iota(iota_t, nt):
    """iota [128,128] viewed as [128, nt, 128] (broadcast middle dim)."""
    a = iota_t[:, :]
    return bass.AP(a.tensor, a.offset, [a.ap[0], [0, nt], a.ap[1]])


def _build_nc(meta, flags):
    NTILES = meta["NTILES"]
    IDXCOLS = meta["IDXCOLS"]
    calls = meta["calls"]
    tilemeta = meta["tilemeta"]
    ln_triv = flags["ln_trivial"]
    fn_triv = flags["fn_trivial"]
    bias_triv = flags["bias_trivial"]
    # the dst dinv may be dropped before the final LN (row-scale-invariant,
    # affine-after-LN unaffected) -- but only when no bias is added between
    # the scale and the LN
    fn_triv_scale = bias_triv[2]

    # default 16KB descriptor carveout: unmerged 5-tile calls reserve only
    # ~41 descriptors per (engine, queue) ring; the freed SBUF goes to a
    # deeper gather pool instead
    nc = bacc.Bacc(num_swdge_queues=4)

    NCALLS = len(calls)
    idx = nc.declare_dram_parameter("idx", [P, IDXCOLS], I16, isOutput=False)
    dstloc = nc.declare_dram_parameter("dstloc", [P, NTILES], BF16, isOutput=False)
    ccnt_in = nc.declare_dram_parameter("ccnt", [1, NCALLS], mybir.dt.uint32, isOutput=False)
    dinv_in = nc.declare_dram_parameter("dinv", [P, BPC], F32, isOutput=False)
    iota_in = nc.declare_dram_parameter("iota", [P, P], BF16, isOutput=False)
    ident_in = nc.declare_dram_parameter("ident", [P, P], F32, isOutput=False)
    ddiag_in = nc.declare_dram_parameter("ddiag", [SH, P], F32, isOutput=False)
    identb_in = nc.declare_dram_parameter("identb", [P, P], BF16, isOutput=False)
    # layer-1 gather tables: host-precomputed dinv*x (bf16), replicated;
    # t1sa/t1sb are this core's own shard halves (for the hs_pre load).
    t1a_in = nc.declare_dram_parameter("t1a", [NPAD // 2, D], BF16, isOutput=False)
    t1b_in = nc.declare_dram_parameter("t1b", [NPAD // 2, D], BF16, isOutput=False)
    HSH_ = SH // 2
    t1sa_in = nc.declare_dram_parameter("t1sa", [HSH_, D], BF16, isOutput=False)
    t1sb_in = nc.declare_dram_parameter("t1sb", [HSH_, D], BF16, isOutput=False)
    w_in = [nc.declare_dram_parameter(f"W{k}", [D, D], F32, isOutput=False) for k in range(3)]
    brep_in = [nc.declare_dram_parameter(f"brep{k}", [P, D], F32, isOutput=False) for k in range(3)]
    lng_in = nc.declare_dram_parameter("lng", [P, D], F32, isOutput=False)
    lnb_in = nc.declare_dram_parameter("lnb", [P, D], F32, isOutput=False)
    fng_in = nc.declare_dram_parameter("fng", [P, D], F32, isOutput=False)
    fnb_in = nc.declare_dram_parameter("fnb", [P, D], F32, isOutput=False)
    out = nc.declare_dram_parameter("out", [SH, D], F32, isOutput=True)

    # ping-pong shard/table buffers: layer 2 uses set 0, layer 3 set 1, so
    # the AllGather for layer k+1 (fired mid-layer-k) never overwrites a
    # table that layer k's remaining gathers still read.
    HSH = SH // 2
    hs_shard = [[nc.dram_tensor(f"hs_shard_{pp_}{h}", [HSH, D], BF16)
                 for h in "ab"] for pp_ in range(2)]
    hs_table = [[nc.dram_tensor(f"hs_table_{pp_}{h}", [NPAD // 2, D], BF16,
                                addr_space="Shared") for h in "ab"] for pp_ in range(2)]
    h1_dram = nc.dram_tensor("h1_dram", [SH, D], F32)

    with tile.TileContext(nc, num_cores=NCORE) as tc:
        with tc.tile_pool(name="persist", bufs=1) as pp, \
             tc.tile_pool(name="stream", bufs=12) as sp, \
             tc.tile_pool(name="gath", bufs=GP_BUFS) as gp, \
             tc.tile_pool(name="epi", bufs=5) as ep, \
             tc.tile_pool(name="dload", bufs=8) as dd, \
             tc.tile_pool(name="psum_agg", bufs=4, space="PSUM") as pa, \
             tc.tile_pool(name="psum_epi", bufs=2, space="PSUM") as pe:

            # ---- persistent loads ----
            from concourse import library_config
            nc.gpsimd.load_library(library_config.mlp)
            idx_sb = pp.tile([P, IDXCOLS], I16)
            c0 = IDXCOLS // 8
            nc.sync.dma_start(idx_sb[:, :c0], idx[:, :c0])
            nc.sync.dma_start(idx_sb[:, c0:], idx[:, c0:])
            dstloc_sb = pp.tile([P, NTILES], BF16)
            nc.sync.dma_start(dstloc_sb[:], dstloc[:])
            ccnt_sb = pp.tile([1, NCALLS], mybir.dt.uint32)
            nc.sync.dma_start(ccnt_sb[:], ccnt_in[:])
            gregs = [nc.gpsimd.alloc_register(f"gather_cnt{i}") for i in range(16)]
            dinv_sb = pp.tile([P, BPC], F32)
            nc.sync.dma_start(dinv_sb[:], dinv_in[:])
            iota_sb = pp.tile([P, P], BF16)
            nc.sync.dma_start(iota_sb[:], iota_in[:])
            ident_sb = pp.tile([P, P], F32)
            nc.sync.dma_start(ident_sb[:], ident_in[:])
            identb_sb = pp.tile([P, P], BF16)
            nc.sync.dma_start(identb_sb[:], identb_in[:])
            w_sb = []
            brep_sb = []
            for k in range(3):
                w = pp.tile([P, D], F32, name=f"w{k}")
                nc.sync.dma_start(w[:], w_in[k][:])
                w_sb.append(w)
                b = pp.tile([P, D], F32, name=f"brep{k}")
                nc.sync.dma_start(b[:], brep_in[k][:])
                brep_sb.append(b)
            lng_sb = pp.tile([P, D], F32)
            nc.sync.dma_start(lng_sb[:], lng_in[:])
            lnb_sb = pp.tile([P, D], F32)
            nc.sync.dma_start(lnb_sb[:], lnb_in[:])
            fng_sb = pp.tile([P, D], F32)
            nc.sync.dma_start(fng_sb[:], fng_in[:])
            fnb_sb = pp.tile([P, D], F32)
            nc.sync.dma_start(fnb_sb[:], fnb_in[:])
            eps_sb = pp.tile([P, 1], F32)
            nc.vector.memset(eps_sb[:], LN_EPS)

            hs_pre = pp.tile([P, BPC * P], BF16)   # next-gather source, node-major chunks
            acc = pp.tile([P, BPC * P], F32)       # aggregation accumulators

            # load hs_pre for layer 1 from this core's precomputed shard
            # halves (t1sa/t1sb), reshaping [bl*128+p, d] -> [p, bl*128+d].
            HB = BPC // 2  # 49 blocks per half
            for half, t1s in ((0, t1sa_in), (1, t1sb_in)):
                a = t1s[:, :]
                src_ap = bass.AP(a.tensor, a.offset, [[D, P], [P * D, HB], [1, D]])
                dst = hs_pre[:, half * HB * P:(half + 1) * HB * P]
                dst_ap = bass.AP(dst.tensor, dst.offset, [dst.ap[0], [P, HB], [1, P]])
                nc.sync.dma_start(dst_ap, src_ap)

            def store_chunk(k, bl):
                blk = slice(bl * P, (bl + 1) * P)
                shards = hs_shard[k % 2]
                if bl < BPC // 2:
                    dst = shards[0][bl * P:(bl + 1) * P, :]
                else:
                    dst = shards[1][(bl - BPC // 2) * P:(bl - BPC // 2 + 1) * P, :]
                nc.sync.dma_start(dst, hs_pre[:, blk])

            def emit_ag(k, which):
                shard, table = hs_shard[k % 2][which], hs_table[k % 2][which]
                nc.gpsimd.collective_compute(
                    "AllGather", mybir.AluOpType.bypass,
                    replica_groups=[list(range(NCORE))],
                    ins=[shard[:].opt()], outs=[table[:].opt()],
                )

            def ln_chunk(h, hsum, g_rep, b_rep, trivial, post_scale=None, out_ap=None):
                """LayerNorm of [128,128] f32 chunk -> new tile (f32).

                hsum ([P,1]) is sum(h) from the producer's accum_out, so no
                DVE reduce is needed; sum(h^2) rides a Square activation's
                accum_out and rstd = Rsqrt(ssq/D + (eps - mu^2)) is one ACT
                op. The final (h-mu)*rstd is one dual-op tensor_scalar;
                post_scale ([P,1] AP) folds an extra per-row factor in."""
                mu = ep.tile([P, 1], F32, tag="mu")
                nc.vector.tensor_scalar_mul(mu[:], hsum[:], 1.0 / D)
                sq = ep.tile([P, D], F32, tag="sq")
                ssq = ep.tile([P, 1], F32, tag="ssq")
                nc.scalar.activation(sq[:], h[:], mybir.ActivationFunctionType.Square,
                                     accum_out=ssq[:])
                mu2 = ep.tile([P, 1], F32, tag="mu2")
                nc.scalar.square(mu2[:], mu[:])
                bias = ep.tile([P, 1], F32, tag="lnbias")  # eps - mu^2
                nc.vector.tensor_scalar(bias[:], mu2[:], -1.0, LN_EPS,
                                        op0=mybir.AluOpType.mult,
                                        op1=mybir.AluOpType.add)
                std = ep.tile([P, 1], F32, tag="std")
                nc.scalar.activation(std[:], ssq[:], mybir.ActivationFunctionType.Sqrt,
                                     bias=bias[:, :1], scale=1.0 / D)
                rstd = ep.tile([P, 1], F32, tag="rstd")
                nc.vector.reciprocal(rstd[:], std[:])
                if post_scale is not None:
                    nc.vector.tensor_mul(rstd[:], rstd[:], post_scale)
                if trivial and out_ap is not None:
                    nc.vector.tensor_scalar(out_ap, h[:], mu[:, :1], rstd[:, :1],
                                            op0=mybir.AluOpType.subtract,
                                            op1=mybir.AluOpType.mult)
                    return None
                norm = ep.tile([P, D], F32, tag="norm")
                nc.vector.tensor_scalar(norm[:], h[:], mu[:, :1], rstd[:, :1],
                                        op0=mybir.AluOpType.subtract,
                                        op1=mybir.AluOpType.mult)
                if not trivial:
                    nc.vector.tensor_mul(norm[:], norm[:], g_rep[:])
                    nc.vector.tensor_add(norm[:], norm[:], b_rep[:])
                return norm

            def epilogue(k, bl):
                """Per-block conv epilogue: dst scale, weight matmul, layer
                tail; stores next layer's hs chunk and re-inits acc[bl]."""
                blk = slice(bl * P, (bl + 1) * P)
                # transpose-and-dst-scale in one matmul: rhs = diag(dinv[blk]).
                # For the last layer the dst scale commutes through W and is
                # cancelled by the final LayerNorm, so a plain transpose does.
                if k == 2 and fn_triv_scale:
                    rhs_t = ident_sb
                else:
                    rhs_t = dd.tile([P, P], F32, tag="ddiag")
                    nc.sync.dma_start(rhs_t[:], ddiag_in[bl * P:(bl + 1) * P, :])
                aggT_p = pe.tile([P, P], F32, tag="aggT", space="PSUM")
                nc.tensor.matmul(aggT_p[:], lhsT=acc[:, blk], rhs=rhs_t[:],
                                 start=True, stop=True)
                aggT = ep.tile([P, P], F32, tag="aggTs")
                nc.scalar.copy(aggT[:], aggT_p[:])
                o_p = pe.tile([P, P], F32, tag="op", space="PSUM")
                nc.tensor.matmul(o_p[:], lhsT=aggT[:], rhs=w_sb[k][:], start=True, stop=True)

                if k == 0:
                    # h = dinv * relu(o): the src-side dinv of the NEXT layer
                    # folded into the ReLU's scale (dinv > 0 commutes with
                    # relu); the LN downstream is scale-invariant, so h1 can
                    # be stored pre-scaled for layer 2's residual add.
                    h = ep.tile([P, D], F32, tag="h")
                    hsum = ep.tile([P, 1], F32, tag="hsum")
                    if bias_triv[k]:
                        nc.scalar.activation(h[:], o_p[:],
                                             mybir.ActivationFunctionType.Relu,
                                             scale=dinv_sb[:, bl:bl + 1],
                                             accum_out=hsum[:])
                    else:
                        o_b = ep.tile([P, D], F32, tag="ob")
                        nc.vector.tensor_add(o_b[:], o_p[:], brep_sb[k][:])
                        nc.scalar.activation(h[:], o_b[:],
                                             mybir.ActivationFunctionType.Relu,
                                             scale=dinv_sb[:, bl:bl + 1],
                                             accum_out=hsum[:])
                    nc.sync.dma_start(h1_dram[bl * P:(bl + 1) * P, :], h[:])
                    if ln_triv:
                        # LN(dinv*x) = LN(x); fold the outer dinv into rstd
                        # and write hs_pre (bf16) straight out of the LN
                        ln_chunk(h, hsum, lng_sb, lnb_sb, True,
                                 post_scale=dinv_sb[:, bl:bl + 1],
                                 out_ap=hs_pre[:, blk])
                    else:
                        ln = ln_chunk(h, hsum, lng_sb, lnb_sb, False)
                        nc.vector.tensor_scalar_mul(hs_pre[:, blk], ln[:],
                                                    dinv_sb[:, bl:bl + 1])
                    store_chunk(k, bl)
                elif k == 1:
                    h = ep.tile([P, D], F32, tag="h")
                    if bias_triv[k]:
                        nc.scalar.activation(h[:], o_p[:],
                                             mybir.ActivationFunctionType.Relu,
                                             scale=dinv_sb[:, bl:bl + 1])
                    else:
                        o_b = ep.tile([P, D], F32, tag="ob")
                        nc.vector.tensor_add(o_b[:], o_p[:], brep_sb[k][:])
                        nc.scalar.activation(h[:], o_b[:],
                                             mybir.ActivationFunctionType.Relu,
                                             scale=dinv_sb[:, bl:bl + 1])
                    h1c = dd.tile([P, D], F32, tag="h1c")
                    nc.sync.dma_start(h1c[:], h1_dram[bl * P:(bl + 1) * P, :])
                    # hs_pre = dinv*(relu(o)+h1) with both terms pre-scaled
                    nc.vector.tensor_tensor(out=hs_pre[:, blk], in0=h[:], in1=h1c[:],
                                            op=mybir.AluOpType.add)
                    store_chunk(k, bl)
                else:
                    o_b = ep.tile([P, D], F32, tag="ob")
                    hsum = ep.tile([P, 1], F32, tag="hsum")
                    if bias_triv[k]:
                        nc.scalar.activation(o_b[:], o_p[:],
                                             mybir.ActivationFunctionType.Copy,
                                             accum_out=hsum[:])
                    else:
                        nc.vector.tensor_add(o_b[:], o_p[:], brep_sb[k][:])
                        nc.scalar.activation(o_b[:], o_b[:],
                                             mybir.ActivationFunctionType.Copy,
                                             accum_out=hsum[:])
                    ln = ln_chunk(o_b, hsum, fng_sb, fnb_sb, fn_triv)
                    nc.sync.dma_start(out[bl * P:(bl + 1) * P, :], ln[:])

            # zero-seed every gather-pool buffer once: trimmed calls leave
            # unfetched lanes holding whatever the buffer had, and 0*NaN
            # from uninitialized SBUF would poison the psum accumulation
            for _ in range(GP_BUFS):
                gseed = gp.tile([P, NIMAX_TILES, D], BF16, tag="g")
                nc.vector.memset(gseed[:], 0.0)

            for k in range(3):  # conv layers
                if k == 0:
                    tabs = (t1a_in, t1b_in)
                else:
                    # layer k reads the set written by layer k-1's epilogue
                    tabs = tuple(hs_table[(k - 1) % 2])

                tcur = 0          # tile cursor
                ccur = 0          # idx col cursor
                qrr = 0
                cur_psum = None
                cur_bl = None
                for ci, (s, groups, nt, off) in enumerate(calls):
                    g = gp.tile([P, NIMAX_TILES, D], BF16, tag="g")
                    tab = tabs[0] if s < 2 else tabs[1]
                    soff = (s % 2) * SUBROWS
                    if ci % 16 == 0:
                        nb = min(16, len(calls) - ci)
                        nc.gpsimd.reg_load(gregs[:nb], ccnt_sb[0:1, ci:ci + nb])
                    n_i = int(meta["nidx"][ci])
                    nc.gpsimd.dma_gather(
                        out_ap=g[:, :nt, :],
                        in_ap=tab[soff:soff + SUBROWS, :],
                        idxs_ap=idx_sb[:, ccur:ccur + (n_i + 15) // 16],
                        num_idxs=n_i, num_idxs_reg=gregs[ci % 16], elem_size=D,
                        # a single packet holds <=64 descriptors per engine
                        # (= 1024 indices); larger merged calls go multi-packet
                        single_packet=(nt * P <= 1024),
                        queue_num=qrr,
                    )
                    qrr = (qrr + 1) % 4
                    S = sp.tile([P, NIMAX_TILES, P], BF16, tag="S")
                    nc.vector.tensor_tensor(
                        out=S[:, :nt, :],
                        in0=dstloc_sb[:, tcur:tcur + nt].to_broadcast([P, nt, P]),
                        in1=_ap3_iota(iota_sb, nt),
                        op=mybir.AluOpType.is_equal)
                    for t in range(nt):
                        bl_, s_, first, last = tilemeta[tcur + t]
                        blk = slice(bl_ * P, (bl_ + 1) * P)
                        if first:
                            cur_psum = pa.tile([P, P], F32, tag="agg", space="PSUM")
                            cur_bl = bl_
                            if s_ == 0:
                                # self-loop term folded into the psum chain
                                nc.tensor.matmul(cur_psum[:], lhsT=identb_sb[:],
                                                 rhs=hs_pre[:, blk],
                                                 start=True, stop=False)
                        assert cur_bl == bl_
                        nc.tensor.matmul(cur_psum[:], lhsT=S[:, t, :], rhs=g[:, t, :],
                                         start=(first and s_ != 0), stop=last)
                        if last:
                            with tc.high_priority(offset=200):
                                if s_ == 0:
                                    nc.vector.tensor_copy(acc[:, blk], cur_psum[:])
                                else:
                                    nc.vector.tensor_add(
                                        acc[:, blk], acc[:, blk], cur_psum[:])
                            if s_ == NSUB - 1:
                                epilogue(k, bl_)
                                if k < 2:
                                    if bl_ == BPC // 2 - 1:
                                        emit_ag(k, 0)
                                    elif bl_ == BPC - 1:
                                        emit_ag(k, 1)
                    tcur += nt
                    ccur += nt * (P // 16)
                assert tcur == NTILES

    nc.finalize()
    return nc


# ---------------- entry point ----------------
def kernel(x, edge_index, W0, b0, W1, b1, W2, b2, ln_g, ln_b, fn_g, fn_b):
    global EXEC_TIME_NS
    x = np.asarray(x, dtype=np.float32)
    meta, data = _preprocess(edge_index)

    flags = {
        "ln_trivial": bool(np.all(np.asarray(ln_g) == 1.0) and np.all(np.asarray(ln_b) == 0.0)),
        "fn_trivial": bool(np.all(np.asarray(fn_g) == 1.0) and np.all(np.asarray(fn_b) == 0.0)),
        "bias_trivial": [bool(np.all(np.asarray(b) == 0.0)) for b in (b0, b1, b2)],
    }
    nc = _build_nc(meta, flags)

    # host-precompute the layer-1 gather table: dinv * x, padded, bf16,
    # in the half-table layout (A = first 6272 rows of every rank's shard).
    HSH = SH // 2
    hs1 = np.zeros((NPAD, D), dtype=np.float32)
    hs1[:N] = x * data["dinv"][:N, None]
    hs1 = hs1.astype(ml_dtypes.bfloat16)
    shards = hs1.reshape(NCORE, SH, D)
    t1a = np.ascontiguousarray(shards[:, :HSH, :].reshape(NPAD // 2, D))
    t1b = np.ascontiguousarray(shards[:, HSH:, :].reshape(NPAD // 2, D))

    iota_arr = np.tile(np.arange(P, dtype=np.float32)[None, :], (P, 1)).astype(ml_dtypes.bfloat16)
    ident_arr = np.eye(P, dtype=np.float32)

    def rep(v):
        return np.tile(np.asarray(v, np.float32)[None, :], (P, 1))

    in_maps = []
    for c in range(NCORE):
        dinv_c = data["dinv"][c * SH:(c + 1) * SH].reshape(BPC, P).T.copy()  # [p, bl]
        ddiag_c = np.zeros((SH, P), dtype=np.float32)
        rows = np.arange(SH)
        ddiag_c[rows, rows % P] = data["dinv"][c * SH:(c + 1) * SH]
        in_maps.append({
            "idx": data["idx"][c],
            "dstloc": data["dstloc"][c],
            "dinv": np.ascontiguousarray(dinv_c),
            "ddiag": ddiag_c,
            "ccnt": data["ccnt"][c][None, :],
            "iota": iota_arr,
            "ident": ident_arr,
            "identb": ident_arr.astype(ml_dtypes.bfloat16),
            "t1a": t1a, "t1b": t1b,
            "t1sa": np.ascontiguousarray(t1a[c * HSH:(c + 1) * HSH]),
            "t1sb": np.ascontiguousarray(t1b[c * HSH:(c + 1) * HSH]),
            "W0": np.asarray(W0, np.float32), "W1": np.asarray(W1, np.float32),
            "W2": np.asarray(W2, np.float32),
            "brep0": rep(b0), "brep1": rep(b1), "brep2": rep(b2),
            "lng": rep(ln_g), "lnb": rep(ln_b),
            "fng": rep(fn_g), "fnb": rep(fn_b),
        })

    profile = bool(os.environ.get("GNN_PROFILE")) and _install_profile_hook()
    res = run_bass_kernel_spmd(nc, in_maps, core_ids=list(range(NCORE)), trace=profile)
    EXEC_TIME_NS = res.exec_time_ns

    out = np.concatenate([res.results[c]["out"] for c in range(NCORE)], axis=0)
    return out[:N]


# revision 77
# speedup vs baseline: 1.0174x; 1.0174x over previous
"""Distributed Trainium2 Bass kernel for a 3-layer GCN (ArithmeticCircuitGNN).

Self-contained: takes full inputs, shards nodes across 8 NeuronCores,
runs the compiled Bass graph via run_bass_kernel_spmd, returns full output.

Math per GCN layer (reference: PyG GCNConv with self-loops):
    out = Dinv (A + I) Dinv (h) W + b        with Dinv = diag(deg^-1/2)
We fold the two Dinv factors into per-node scalings:
    hs   = dinv * h                 (source-side, before gather)
    agg  = (A + I) hs               (gather + one-hot matmul scatter-add)
    out  = (dinv * agg) W + b       (dst-side scale, then weight matmul)

Perf structure (2.39 ms vs 5.64 ms baseline; bottleneck is GPSIMD/Q7
SWDGE descriptor generation at ~1.7 us per 640-index gather call):
  - layer-1 gather tables are host-precomputed (dinv*x in bf16) and fed as
    input params: no conv-pre phase, no layer-1 AllGather, ~25 us startup.
  - gather calls are aligned to (dst-block, sub-table) groups; each core's
    trailing pad lanes carry idx=-1 (dropped by the ucode's trailing-
    negative trim) with num_idxs_reg loaded per-core from SBUF via
    batched reg_load, so ring reservation == emitted descriptors. This
    cuts ~20% of the random 256B HBM reads and keeps calls single-packet
    (<= 64 descriptors per engine).
  - stale-lane safety: trimmed lanes leave old SBUF bytes in the gather
    tile (masked by dstloc=200 -> S=0), so every pool buffer is memset to
    zero once at startup - 0 * NaN from uninitialized SBUF would
    otherwise poison the psum accumulation.
  - tile stream is dst-half-major then sub-major: per-block epilogues run
    inside each half's last sub pass, and each AllGather fires mid-layer
    with ~half a layer of gather stream to hide behind; table/shard
    buffers ping-pong between layers so an early AllGather never
    overwrites a table the current layer still reads.
  - the self-loop term enters each block's first psum chain as an
    identity matmul; the dst-side dinv rides the transpose matmul as a
    host-built diagonal (dropped entirely for layer 3: the final
    LayerNorm is row-scale-invariant); the next layer's src-side dinv is
    folded into the ReLU scale (h1 residual stored pre-scaled) and into
    the LayerNorm's rstd; LN computes var = E[h^2]-mu^2 and emits
    (h-mu)*rstd as one dual-op tensor_scalar, straight to bf16 hs_pre.
"""

import contextlib
import ctypes
import os
import sys
import types

import numpy as np
import ml_dtypes

import concourse.bass as bass
import concourse.mybir as mybir
import concourse.tile as tile
from concourse import bacc
from concourse.bass_utils import run_bass_kernel_spmd

# ---------------- problem constants (hardcoded per spec) ----------------
N = 100000
E = 1600000
D = 128
P = 128
NCORE = 8
BPC = 98                 # dst blocks of 128 nodes per core
SH = BPC * P             # 12544 nodes per core shard
NPAD = NCORE * SH        # 100352 padded node count
NSUB = 4                 # sub-tables (int16 index reach)
SUBROWS = NPAD // NSUB   # 25088 rows per sub-table
NIMAX_TILES = 5          # max tiles per dma_gather call (= largest group)
MERGE_CALLS = False      # merged calls need multi-packet descriptors, which
                         # drain slower than the saved per-call fixed cost
PADLOC = 200.0           # dstloc value for padding lanes (> 127)
GP_BUFS = 28             # gather-pool depth
NOTRIM_CALLS = 0         # gather-pool buffers are memset once at startup, so
                         # every call can trim its pad lanes (idx=-1); trimmed
                         # lanes then read zeros (finite), never stale NaN bits
LN_EPS = 1e-5

BF16 = mybir.dt.bfloat16
F32 = mybir.dt.float32
I16 = mybir.dt.int16

EXEC_TIME_NS = None      # set by kernel() when profiling is enabled


# ---------------- axon NTFF profile hook (optional) ----------------
def _install_profile_hook():
    so_path = "/opt/axon/libaxon_pjrt.so"
    if "antenv.axon_hooks" in sys.modules:
        return True
    try:
        lib = ctypes.CDLL(so_path)
        if not hasattr(lib, "axon_start_nrt_profile"):
            return False
        lib.axon_start_nrt_profile.argtypes = [ctypes.POINTER(ctypes.c_int64), ctypes.c_size_t]
        lib.axon_start_nrt_profile.restype = ctypes.c_int64
        lib.axon_stop_nrt_profile.argtypes = [ctypes.c_char_p]
        lib.axon_stop_nrt_profile.restype = ctypes.c_int64

        @contextlib.contextmanager
        def _hook(output_dir, device_ids):
            import jax
            jax.devices()
            if device_ids:
                ids = (ctypes.c_int64 * len(device_ids))(*device_ids)
                rc = lib.axon_start_nrt_profile(ids, len(device_ids))
            else:
                rc = lib.axon_start_nrt_profile(None, 0)
            if rc != 0:
                raise RuntimeError(f"axon_start_nrt_profile rc={rc}")
            try:
                yield
            finally:
                n = lib.axon_stop_nrt_profile(str(output_dir).encode())
                if n < 0:
                    raise RuntimeError(f"axon_stop_nrt_profile rc={n}")

        mod = types.ModuleType("antenv.axon_hooks")
        mod.get_axon_ntff_profile_hook = lambda: _hook
        mod.set_axon_ntff_profile_hook = lambda h: None
        sys.modules["antenv.axon_hooks"] = mod

        import concourse.bass_utils as bu
        bu.upload_artifacts = lambda tmpdir: f"local:{tmpdir}"
        return True
    except Exception:
        return False


# ---------------- host-side graph preprocessing ----------------
def _preprocess(edge_index):
    src = np.asarray(edge_index[0], dtype=np.int64)
    dst = np.asarray(edge_index[1], dtype=np.int64)

    deg = np.bincount(dst, minlength=NPAD).astype(np.float64) + 1.0
    dinv = (1.0 / np.sqrt(deg)).astype(np.float32)  # padding nodes -> 1.0

    # table row of node g: owner rank halves are concatenated into two
    # half-tables (A = first 6272 rows of every rank, B = second half).
    HSH = SH // 2
    r_own = src // SH
    off = src % SH
    half = off // HSH
    lrow = r_own * HSH + (off % HSH)          # row within half-table
    sub = half * 2 + lrow // SUBROWS          # 0..3
    srcloc_all = lrow % SUBROWS
    gblk = dst // P                           # global dst block 0..781
    key = gblk * NSUB + sub
    order = np.argsort(key, kind="stable")
    src_s, dst_s, key_s = src[order], dst[order], key[order]
    srcloc_s = srcloc_all[order]

    NKEY = NCORE * BPC * NSUB                 # 784*4 (incl. empty tail blocks)
    cnt = np.bincount(key_s, minlength=NKEY)
    # per (core, local block, sub) counts; blocks 782/783 are zero
    cnt_cbs = cnt.reshape(NCORE, BPC, NSUB)
    T_u = np.ceil(cnt_cbs / P).astype(np.int64).max(axis=0)  # [BPC, NSUB]

    # tile stream: for dst-block half: for s in 0..3: for bl in half.
    # Half-major ordering closes blocks 0-48 (shard A) at ~50% of the layer
    # so the next layer's AllGather A can fire mid-layer; sub-major within a
    # half keeps table_b unneeded until ~25% into the stream.
    HB = BPC // 2 + BPC % 2                   # 49 blocks in the first half
    blorder = [(s, bl) for half in (range(HB), range(HB, BPC))
               for s in range(NSUB) for bl in half]
    NTILES = int(T_u.sum())

    # column base of each (s, bl) group in the tile stream
    group_base = np.zeros((NSUB, BPC), dtype=np.int64)
    cur = 0
    for s, bl in blorder:
        group_base[s, bl] = cur
        cur += int(T_u[bl, s])
    assert cur == NTILES

    # gather calls: merge same-s stream-adjacent groups per call (they read
    # the same sub-table), amortizing the ~1.5us Q7 per-call fixed cost.
    # Only the call-trailing pads (the last group's tail) can be dropped by
    # the ucode's trailing-negative trim; earlier merged groups' pads stay
    # idx=0 and are fetched (cheap, the drain has slack).
    # calls: list of (s, groups, ntiles) with groups = [(bl, T), ...].
    glist = [(s, bl, int(T_u[bl, s])) for s, bl in blorder if T_u[bl, s] > 0]
    calls = []                                # (s, groups, ntiles, off_tiles)
    i = 0
    while i < len(glist):
        s, bl, T = glist[i]
        if T > NIMAX_TILES:                   # oversized group: chunk it
            done = 0
            while done < T:
                ch = min(NIMAX_TILES, T - done)
                calls.append((s, [(bl, ch)], ch, done))
                done += ch
            i += 1
            continue
        groups = [(bl, T)]
        nt = T
        while (MERGE_CALLS and i + 1 < len(glist) and glist[i + 1][0] == s
               and nt + glist[i + 1][2] <= NIMAX_TILES):
            i += 1
            groups.append((glist[i][1], glist[i][2]))
            nt += glist[i][2]
        calls.append((s, groups, nt, 0))
        i += 1

    # tile meta: (bl, s, first_of_group, last_of_group) in stream order
    tilemeta = []
    for s, bl in blorder:
        T = int(T_u[bl, s])
        for t in range(T):
            tilemeta.append((bl, s, t == 0, t == T - 1))

    # per-core edge placement
    starts = np.zeros(NKEY + 1, dtype=np.int64)
    starts[1:] = np.cumsum(cnt)
    rank = np.arange(len(src_s)) - np.repeat(starts[:-1], cnt)

    core_e = gblk[order] // BPC               # owning core of each (sorted) edge
    bl_e = gblk[order] % BPC
    sub_e = key_s % NSUB
    pos = group_base[sub_e, bl_e] * P + rank  # slot in the core's edge stream

    src_local = srcloc_s.astype(np.int16)
    dst_local = (dst_s - (core_e * SH + bl_e * P)).astype(np.float32)

    srcbuf = np.full((NCORE, NTILES * P), -1, dtype=np.int16)
    dstbuf = np.full((NCORE, NTILES * P), PADLOC, dtype=np.float32)
    for c in range(NCORE):
        m = core_e == c
        srcbuf[c, pos[m]] = src_local[m]
        dstbuf[c, pos[m]] = dst_local[m]

    # dstloc sbuf layout: [p, tile]
    dstloc = dstbuf.reshape(NCORE, NTILES, P).transpose(0, 2, 1)  # [c, 128, NTILES]

    # per-core per-call valid-index counts: the gather's num_idxs_reg must
    # equal the number of non-negative indices (the ucode's ring-space
    # reservation and trigger counts are reg-based while descriptor emission
    # is trimmed-data-based; they must agree or stale descriptors fire).
    # Within a call, all groups but the last count full T*128 lanes (their
    # pads are idx=0); the last group is trimmed to this core's edge count.
    ccnt = np.zeros((NCORE, len(calls)), dtype=np.uint32)
    tc0 = 0
    for ci, (s, groups, nt, off) in enumerate(calls):
        if ci < NOTRIM_CALLS:
            ccnt[:, ci] = nt * P
        else:
            bl_last, T_last = groups[-1]
            full = (nt - T_last) * P
            ccnt[:, ci] = full + np.clip(
                cnt_cbs[:, bl_last, s] - off * P, 0, T_last * P)
        tc0 += nt
    assert tc0 == NTILES
    # exact per-call num_idxs (shared immediate): the Q7 widen loop runs
    # ceil(num_idxs/16) iterations, so shave it to the max core's count
    nidx = ((ccnt.max(axis=0).astype(np.int64) + 15) // 16 * 16)

    # pad-lane idx values: -1 (trimmed) only in each call's LAST group;
    # earlier merged groups' pads become 0 (fetched, masked by dstloc=200).
    tc0 = 0
    for ci, (s, groups, nt, off) in enumerate(calls):
        bl_last, T_last = groups[-1]
        lo = tc0 * P
        hi_nontrim = (tc0 + nt - T_last) * P
        if ci < NOTRIM_CALLS:
            hi_nontrim = (tc0 + nt) * P
        seg = srcbuf[:, lo:hi_nontrim]
        seg[seg < 0] = 0
        tc0 += nt
    assert tc0 == NTILES

    # idx16 layout per call: element i -> [i%16, base + i//16], replicated x8.
    idxcols = NTILES * (P // 16)
    idxbuf = np.zeros((NCORE, 16, idxcols), dtype=np.int16)
    tc = 0
    colc = 0
    for ci, (s, groups, nt, off) in enumerate(calls):
        n = nt * P
        blk = srcbuf[:, tc * P:tc * P + n].reshape(NCORE, n // 16, 16)
        idxbuf[:, :, colc:colc + n // 16] = blk.transpose(0, 2, 1)
        tc += nt
        colc += n // 16
    assert tc == NTILES and colc == idxcols
    idx_arr = np.tile(idxbuf, (1, 8, 1))      # [c, 128, idxcols]

    meta = {
        "NTILES": NTILES,
        "IDXCOLS": idxcols,
        "calls": calls,
        "tilemeta": tilemeta,
        "nidx": nidx,
    }
    data = {
        "idx": idx_arr,
        "dstloc": dstloc.astype(ml_dtypes.bfloat16),
        "dinv": dinv,
        "ccnt": ccnt,
    }
    return meta, data


# ---------------- device graph ----------------
def _ap3_iota(iota_t, nt):
    """iota [128,128] viewed as [128, nt, 128] (broadcast middle dim)."""
    a = iota_t[:, :]
    return bass.AP(a.tensor, a.offset, [a.ap[0], [0, nt], a.ap[1]])


def _build_nc(meta, flags):
    NTILES = meta["NTILES"]
    IDXCOLS = meta["IDXCOLS"]
    calls = meta["calls"]
    tilemeta = meta["tilemeta"]
    ln_triv = flags["ln_trivial"]
    fn_triv = flags["fn_trivial"]
    bias_triv = flags["bias_trivial"]
    # the dst dinv may be dropped before the final LN (row-scale-invariant,
    # affine-after-LN unaffected) -- but only when no bias is added between
    # the scale and the LN
    fn_triv_scale = bias_triv[2]

    # 32KB descriptor carveout: deeper per-queue rings keep more gather
    # calls in flight (a 5-tile call reserves ~41 descriptors per ring)
    nc = bacc.Bacc(num_swdge_queues=4, dynamic_dma_scratch_size=32768)

    NCALLS = len(calls)
    idx = nc.declare_dram_parameter("idx", [P, IDXCOLS], I16, isOutput=False)
    dstloc = nc.declare_dram_parameter("dstloc", [P, NTILES], BF16, isOutput=False)
    ccnt_in = nc.declare_dram_parameter("ccnt", [1, NCALLS], mybir.dt.uint32, isOutput=False)
    dinv_in = nc.declare_dram_parameter("dinv", [P, BPC], F32, isOutput=False)
    iota_in = nc.declare_dram_parameter("iota", [P, P], BF16, isOutput=False)
    ident_in = nc.declare_dram_parameter("ident", [P, P], F32, isOutput=False)
    ddiag_in = nc.declare_dram_parameter("ddiag", [SH, P], F32, isOutput=False)
    identb_in = nc.declare_dram_parameter("identb", [P, P], BF16, isOutput=False)
    # layer-1 gather tables: host-precomputed dinv*x (bf16), replicated;
    # t1sa/t1sb are this core's own shard halves (for the hs_pre load).
    t1a_in = nc.declare_dram_parameter("t1a", [NPAD // 2, D], BF16, isOutput=False)
    t1b_in = nc.declare_dram_parameter("t1b", [NPAD // 2, D], BF16, isOutput=False)
    HSH_ = SH // 2
    t1sa_in = nc.declare_dram_parameter("t1sa", [HSH_, D], BF16, isOutput=False)
    t1sb_in = nc.declare_dram_parameter("t1sb", [HSH_, D], BF16, isOutput=False)
    w_in = [nc.declare_dram_parameter(f"W{k}", [D, D], F32, isOutput=False) for k in range(3)]
    brep_in = [nc.declare_dram_parameter(f"brep{k}", [P, D], F32, isOutput=False) for k in range(3)]
    lng_in = nc.declare_dram_parameter("lng", [P, D], F32, isOutput=False)
    lnb_in = nc.declare_dram_parameter("lnb", [P, D], F32, isOutput=False)
    fng_in = nc.declare_dram_parameter("fng", [P, D], F32, isOutput=False)
    fnb_in = nc.declare_dram_parameter("fnb", [P, D], F32, isOutput=False)
    out = nc.declare_dram_parameter("out", [SH, D], F32, isOutput=True)

    # ping-pong shard/table buffers: layer 2 uses set 0, layer 3 set 1, so
    # the AllGather for layer k+1 (fired mid-layer-k) never overwrites a
    # table that layer k's remaining gathers still read.
    HSH = SH // 2
    hs_shard = [[nc.dram_tensor(f"hs_shard_{pp_}{h}", [HSH, D], BF16)
                 for h in "ab"] for pp_ in range(2)]
    hs_table = [[nc.dram_tensor(f"hs_table_{pp_}{h}", [NPAD // 2, D], BF16,
                                addr_space="Shared") for h in "ab"] for pp_ in range(2)]
    h1_dram = nc.dram_tensor("h1_dram", [SH, D], F32)

    with tile.TileContext(nc, num_cores=NCORE) as tc:
        with tc.tile_pool(name="persist", bufs=1) as pp, \
             tc.tile_pool(name="stream", bufs=12) as sp, \
             tc.tile_pool(name="gath", bufs=GP_BUFS) as gp, \
             tc.tile_pool(name="epi", bufs=5) as ep, \
             tc.tile_pool(name="dload", bufs=8) as dd, \
             tc.tile_pool(name="psum_agg", bufs=4, space="PSUM") as pa, \
             tc.tile_pool(name="psum_epi", bufs=2, space="PSUM") as pe:

            # ---- persistent loads ----
            from concourse import library_config
            nc.gpsimd.load_library(library_config.mlp)
            idx_sb = pp.tile([P, IDXCOLS], I16)
            c0 = IDXCOLS // 8
            nc.sync.dma_start(idx_sb[:, :c0], idx[:, :c0])
            nc.sync.dma_start(idx_sb[:, c0:], idx[:, c0:])
            dstloc_sb = pp.tile([P, NTILES], BF16)
            nc.sync.dma_start(dstloc_sb[:], dstloc[:])
            ccnt_sb = pp.tile([1, NCALLS], mybir.dt.uint32)
            nc.sync.dma_start(ccnt_sb[:], ccnt_in[:])
            gregs = [nc.gpsimd.alloc_register(f"gather_cnt{i}") for i in range(16)]
            dinv_sb = pp.tile([P, BPC], F32)
            nc.sync.dma_start(dinv_sb[:], dinv_in[:])
            iota_sb = pp.tile([P, P], BF16)
            nc.sync.dma_start(iota_sb[:], iota_in[:])
            ident_sb = pp.tile([P, P], F32)
            nc.sync.dma_start(ident_sb[:], ident_in[:])
            identb_sb = pp.tile([P, P], BF16)
            nc.sync.dma_start(identb_sb[:], identb_in[:])
            w_sb = []
            brep_sb = []
            for k in range(3):
                w = pp.tile([P, D], F32, name=f"w{k}")
                nc.sync.dma_start(w[:], w_in[k][:])
                w_sb.append(w)
                b = pp.tile([P, D], F32, name=f"brep{k}")
                nc.sync.dma_start(b[:], brep_in[k][:])
                brep_sb.append(b)
            lng_sb = pp.tile([P, D], F32)
            nc.sync.dma_start(lng_sb[:], lng_in[:])
            lnb_sb = pp.tile([P, D], F32)
            nc.sync.dma_start(lnb_sb[:], lnb_in[:])
            fng_sb = pp.tile([P, D], F32)
            nc.sync.dma_start(fng_sb[:], fng_in[:])
            fnb_sb = pp.tile([P, D], F32)
            nc.sync.dma_start(fnb_sb[:], fnb_in[:])
            eps_sb = pp.tile([P, 1], F32)
            nc.vector.memset(eps_sb[:], LN_EPS)

            hs_pre = pp.tile([P, BPC * P], BF16)   # next-gather source, node-major chunks
            acc = pp.tile([P, BPC * P], F32)       # aggregation accumulators

            # load hs_pre for layer 1 from this core's precomputed shard
            # halves (t1sa/t1sb), reshaping [bl*128+p, d] -> [p, bl*128+d].
            HB = BPC // 2  # 49 blocks per half
            for half, t1s in ((0, t1sa_in), (1, t1sb_in)):
                a = t1s[:, :]
                src_ap = bass.AP(a.tensor, a.offset, [[D, P], [P * D, HB], [1, D]])
                dst = hs_pre[:, half * HB * P:(half + 1) * HB * P]
                dst_ap = bass.AP(dst.tensor, dst.offset, [dst.ap[0], [P, HB], [1, P]])
                nc.sync.dma_start(dst_ap, src_ap)

            def store_chunk(k, bl):
                blk = slice(bl * P, (bl + 1) * P)
                shards = hs_shard[k % 2]
                if bl < BPC // 2:
                    dst = shards[0][bl * P:(bl + 1) * P, :]
                else:
                    dst = shards[1][(bl - BPC // 2) * P:(bl - BPC // 2 + 1) * P, :]
                nc.sync.dma_start(dst, hs_pre[:, blk])

            def emit_ag(k, which):
                shard, table = hs_shard[k % 2][which], hs_table[k % 2][which]
                nc.gpsimd.collective_compute(
                    "AllGather", mybir.AluOpType.bypass,
                    replica_groups=[list(range(NCORE))],
                    ins=[shard[:].opt()], outs=[table[:].opt()],
                )

            def ln_chunk(h, hsum, g_rep, b_rep, trivial, post_scale=None, out_ap=None):
                """LayerNorm of [128,128] f32 chunk -> new tile (f32).

                hsum ([P,1]) is sum(h) from the producer's accum_out, so no
                DVE reduce is needed; sum(h^2) rides a Square activation's
                accum_out and rstd = Rsqrt(ssq/D + (eps - mu^2)) is one ACT
                op. The final (h-mu)*rstd is one dual-op tensor_scalar;
                post_scale ([P,1] AP) folds an extra per-row factor in."""
                mu = ep.tile([P, 1], F32, tag="mu")
                nc.vector.tensor_scalar_mul(mu[:], hsum[:], 1.0 / D)
                sq = ep.tile([P, D], F32, tag="sq")
                ssq = ep.tile([P, 1], F32, tag="ssq")
                nc.scalar.activation(sq[:], h[:], mybir.ActivationFunctionType.Square,
                                     accum_out=ssq[:])
                mu2 = ep.tile([P, 1], F32, tag="mu2")
                nc.scalar.square(mu2[:], mu[:])
                bias = ep.tile([P, 1], F32, tag="lnbias")  # eps - mu^2
                nc.vector.tensor_scalar(bias[:], mu2[:], -1.0, LN_EPS,
                                        op0=mybir.AluOpType.mult,
                                        op1=mybir.AluOpType.add)
                std = ep.tile([P, 1], F32, tag="std")
                nc.scalar.activation(std[:], ssq[:], mybir.ActivationFunctionType.Sqrt,
                                     bias=bias[:, :1], scale=1.0 / D)
                rstd = ep.tile([P, 1], F32, tag="rstd")
                nc.vector.reciprocal(rstd[:], std[:])
                if post_scale is not None:
                    nc.vector.tensor_mul(rstd[:], rstd[:], post_scale)
                if trivial and out_ap is not None:
                    nc.vector.tensor_scalar(out_ap, h[:], mu[:, :1], rstd[:, :1],
                                            op0=mybir.AluOpType.subtract,
                                            op1=mybir.AluOpType.mult)
                    return None
                norm = ep.tile([P, D], F32, tag="norm")
                nc.vector.tensor_scalar(norm[:], h[:], mu[:, :1], rstd[:, :1],
                                        op0=mybir.AluOpType.subtract,
                                        op1=mybir.AluOpType.mult)
                if not trivial:
                    nc.vector.tensor_mul(norm[:], norm[:], g_rep[:])
                    nc.vector.tensor_add(norm[:], norm[:], b_rep[:])
                return norm

            def epilogue(k, bl):
                """Per-block conv epilogue: dst scale, weight matmul, layer
                tail; stores next layer's hs chunk and re-inits acc[bl]."""
                blk = slice(bl * P, (bl + 1) * P)
                # transpose-and-dst-scale in one matmul: rhs = diag(dinv[blk]).
                # For the last layer the dst scale commutes through W and is
                # cancelled by the final LayerNorm, so a plain transpose does.
                if k == 2 and fn_triv_scale:
                    rhs_t = ident_sb
                else:
                    rhs_t = dd.tile([P, P], F32, tag="ddiag")
                    nc.sync.dma_start(rhs_t[:], ddiag_in[bl * P:(bl + 1) * P, :])
                aggT_p = pe.tile([P, P], F32, tag="aggT", space="PSUM")
                nc.tensor.matmul(aggT_p[:], lhsT=acc[:, blk], rhs=rhs_t[:],
                                 start=True, stop=True)
                aggT = ep.tile([P, P], F32, tag="aggTs")
                nc.scalar.copy(aggT[:], aggT_p[:])
                o_p = pe.tile([P, P], F32, tag="op", space="PSUM")
                nc.tensor.matmul(o_p[:], lhsT=aggT[:], rhs=w_sb[k][:], start=True, stop=True)

                if k == 0:
                    # h = dinv * relu(o): the src-side dinv of the NEXT layer
                    # folded into the ReLU's scale (dinv > 0 commutes with
                    # relu); the LN downstream is scale-invariant, so h1 can
                    # be stored pre-scaled for layer 2's residual add.
                    h = ep.tile([P, D], F32, tag="h")
                    hsum = ep.tile([P, 1], F32, tag="hsum")
                    if bias_triv[k]:
                        nc.scalar.activation(h[:], o_p[:],
                                             mybir.ActivationFunctionType.Relu,
                                             scale=dinv_sb[:, bl:bl + 1],
                                             accum_out=hsum[:])
                    else:
                        o_b = ep.tile([P, D], F32, tag="ob")
                        nc.vector.tensor_add(o_b[:], o_p[:], brep_sb[k][:])
                        nc.scalar.activation(h[:], o_b[:],
                                             mybir.ActivationFunctionType.Relu,
                                             scale=dinv_sb[:, bl:bl + 1],
                                             accum_out=hsum[:])
                    nc.sync.dma_start(h1_dram[bl * P:(bl + 1) * P, :], h[:])
                    if ln_triv:
                        # LN(dinv*x) = LN(x); fold the outer dinv into rstd
                        # and write hs_pre (bf16) straight out of the LN
                        ln_chunk(h, hsum, lng_sb, lnb_sb, True,
                                 post_scale=dinv_sb[:, bl:bl + 1],
                                 out_ap=hs_pre[:, blk])
                    else:
                        ln = ln_chunk(h, hsum, lng_sb, lnb_sb, False)
                        nc.vector.tensor_scalar_mul(hs_pre[:, blk], ln[:],
                                                    dinv_sb[:, bl:bl + 1])
                    store_chunk(k, bl)
                elif k == 1:
                    h = ep.tile([P, D], F32, tag="h")
                    if bias_triv[k]:
                        nc.scalar.activation(h[:], o_p[:],
                                             mybir.ActivationFunctionType.Relu,
                                             scale=dinv_sb[:, bl:bl + 1])
                    else:
                        o_b = ep.tile([P, D], F32, tag="ob")
                        nc.vector.tensor_add(o_b[:], o_p[:], brep_sb[k][:])
                        nc.scalar.activation(h[:], o_b[:],
                                             mybir.ActivationFunctionType.Relu,
                                             scale=dinv_sb[:, bl:bl + 1])
                    h1c = dd.tile([P, D], F32, tag="h1c")
                    nc.sync.dma_start(h1c[:], h1_dram[bl * P:(bl + 1) * P, :])
                    # hs_pre = dinv*(relu(o)+h1) with both terms pre-scaled
                    nc.vector.tensor_tensor(out=hs_pre[:, blk], in0=h[:], in1=h1c[:],
                                            op=mybir.AluOpType.add)
                    store_chunk(k, bl)
                else:
                    o_b = ep.tile([P, D], F32, tag="ob")
                    hsum = ep.tile([P, 1], F32, tag="hsum")
                    if bias_triv[k]:
                        nc.scalar.activation(o_b[:], o_p[:],
                                             mybir.ActivationFunctionType.Copy,
                                             accum_out=hsum[:])
                    else:
                        nc.vector.tensor_add(o_b[:], o_p[:], brep_sb[k][:])
                        nc.scalar.activation(o_b[:], o_b[:],
                                             mybir.ActivationFunctionType.Copy,
                                             accum_out=hsum[:])
                    ln = ln_chunk(o_b, hsum, fng_sb, fnb_sb, fn_triv)
                    nc.sync.dma_start(out[bl * P:(bl + 1) * P, :], ln[:])

            # zero-seed every gather-pool buffer once: trimmed calls leave
            # unfetched lanes holding whatever the buffer had, and 0*NaN
            # from uninitialized SBUF would poison the psum accumulation
            for _ in range(GP_BUFS):
                gseed = gp.tile([P, NIMAX_TILES, D], BF16, tag="g")
                nc.vector.memset(gseed[:], 0.0)

            for k in range(3):  # conv layers
                if k == 0:
                    tabs = (t1a_in, t1b_in)
                else:
                    # layer k reads the set written by layer k-1's epilogue
                    tabs = tuple(hs_table[(k - 1) % 2])

                tcur = 0          # tile cursor
                ccur = 0          # idx col cursor
                qrr = 0
                cur_psum = None
                cur_bl = None
                for ci, (s, groups, nt, off) in enumerate(calls):
                    g = gp.tile([P, NIMAX_TILES, D], BF16, tag="g")
                    tab = tabs[0] if s < 2 else tabs[1]
                    soff = (s % 2) * SUBROWS
                    if ci % 16 == 0:
                        nb = min(16, len(calls) - ci)
                        nc.gpsimd.reg_load(gregs[:nb], ccnt_sb[0:1, ci:ci + nb])
                    n_i = int(meta["nidx"][ci])
                    nc.gpsimd.dma_gather(
                        out_ap=g[:, :nt, :],
                        in_ap=tab[soff:soff + SUBROWS, :],
                        idxs_ap=idx_sb[:, ccur:ccur + (n_i + 15) // 16],
                        num_idxs=n_i, num_idxs_reg=gregs[ci % 16], elem_size=D,
                        # a single packet holds <=64 descriptors per engine
                        # (= 1024 indices); larger merged calls go multi-packet
                        single_packet=(nt * P <= 1024),
                        queue_num=qrr,
                    )
                    qrr = (qrr + 1) % 4
                    S = sp.tile([P, NIMAX_TILES, P], BF16, tag="S")
                    nc.vector.tensor_tensor(
                        out=S[:, :nt, :],
                        in0=dstloc_sb[:, tcur:tcur + nt].to_broadcast([P, nt, P]),
                        in1=_ap3_iota(iota_sb, nt),
                        op=mybir.AluOpType.is_equal)
                    for t in range(nt):
                        bl_, s_, first, last = tilemeta[tcur + t]
                        blk = slice(bl_ * P, (bl_ + 1) * P)
                        if first:
                            cur_psum = pa.tile([P, P], F32, tag="agg", space="PSUM")
                            cur_bl = bl_
                            if s_ == 0:
                                # self-loop term folded into the psum chain
                                nc.tensor.matmul(cur_psum[:], lhsT=identb_sb[:],
                                                 rhs=hs_pre[:, blk],
                                                 start=True, stop=False)
                        assert cur_bl == bl_
                        nc.tensor.matmul(cur_psum[:], lhsT=S[:, t, :], rhs=g[:, t, :],
                                         start=(first and s_ != 0), stop=last)
                        if last:
                            with tc.high_priority(offset=200):
                                if s_ == 0:
                                    nc.vector.tensor_copy(acc[:, blk], cur_psum[:])
                                else:
                                    nc.vector.tensor_add(
                                        acc[:, blk], acc[:, blk], cur_psum[:])
                            if s_ == NSUB - 1:
                                epilogue(k, bl_)
                                if k < 2:
                                    if bl_ == BPC // 2 - 1:
                                        emit_ag(k, 0)
                                    elif bl_ == BPC - 1:
                                        emit_ag(k, 1)
                    tcur += nt
                    ccur += nt * (P // 16)
                assert tcur == NTILES

    nc.finalize()
    return nc


# ---------------- entry point ----------------
def kernel(x, edge_index, W0, b0, W1, b1, W2, b2, ln_g, ln_b, fn_g, fn_b):
    global EXEC_TIME_NS
    x = np.asarray(x, dtype=np.float32)
    meta, data = _preprocess(edge_index)

    flags = {
        "ln_trivial": bool(np.all(np.asarray(ln_g) == 1.0) and np.all(np.asarray(ln_b) == 0.0)),
        "fn_trivial": bool(np.all(np.asarray(fn_g) == 1.0) and np.all(np.asarray(fn_b) == 0.0)),
        "bias_trivial": [bool(np.all(np.asarray(b) == 0.0)) for b in (b0, b1, b2)],
    }
    nc = _build_nc(meta, flags)

    # host-precompute the layer-1 gather table: dinv * x, padded, bf16,
    # in the half-table layout (A = first 6272 rows of every rank's shard).
    HSH = SH // 2
    hs1 = np.zeros((NPAD, D), dtype=np.float32)
    hs1[:N] = x * data["dinv"][:N, None]
    hs1 = hs1.astype(ml_dtypes.bfloat16)
    shards = hs1.reshape(NCORE, SH, D)
    t1a = np.ascontiguousarray(shards[:, :HSH, :].reshape(NPAD // 2, D))
    t1b = np.ascontiguousarray(shards[:, HSH:, :].reshape(NPAD // 2, D))

    iota_arr = np.tile(np.arange(P, dtype=np.float32)[None, :], (P, 1)).astype(ml_dtypes.bfloat16)
    ident_arr = np.eye(P, dtype=np.float32)

    def rep(v):
        return np.tile(np.asarray(v, np.float32)[None, :], (P, 1))

    in_maps = []
    for c in range(NCORE):
        dinv_c = data["dinv"][c * SH:(c + 1) * SH].reshape(BPC, P).T.copy()  # [p, bl]
        ddiag_c = np.zeros((SH, P), dtype=np.float32)
        rows = np.arange(SH)
        ddiag_c[rows, rows % P] = data["dinv"][c * SH:(c + 1) * SH]
        in_maps.append({
            "idx": data["idx"][c],
            "dstloc": data["dstloc"][c],
            "dinv": np.ascontiguousarray(dinv_c),
            "ddiag": ddiag_c,
            "ccnt": data["ccnt"][c][None, :],
            "iota": iota_arr,
            "ident": ident_arr,
            "identb": ident_arr.astype(ml_dtypes.bfloat16),
            "t1a": t1a, "t1b": t1b,
            "t1sa": np.ascontiguousarray(t1a[c * HSH:(c + 1) * HSH]),
            "t1sb": np.ascontiguousarray(t1b[c * HSH:(c + 1) * HSH]),
            "W0": np.asarray(W0, np.float32), "W1": np.asarray(W1, np.float32),
            "W2": np.asarray(W2, np.float32),
            "brep0": rep(b0), "brep1": rep(b1), "brep2": rep(b2),
            "lng": rep(ln_g), "lnb": rep(ln_b),
            "fng": rep(fn_g), "fnb": rep(fn_b),
        })

    profile = bool(os.environ.get("GNN_PROFILE")) and _install_profile_hook()
    res = run_bass_kernel_spmd(nc, in_maps, core_ids=list(range(NCORE)), trace=profile)
    EXEC_TIME_NS = res.exec_time_ns

    out = np.concatenate([res.results[c]["out"] for c in range(NCORE)], axis=0)
    return out[:N]


# revision 78
# speedup vs baseline: 1.0290x; 1.0114x over previous
"""Distributed Trainium2 Bass kernel for a 3-layer GCN (ArithmeticCircuitGNN).

Self-contained: takes full inputs, shards nodes across 8 NeuronCores,
runs the compiled Bass graph via run_bass_kernel_spmd, returns full output.

Math per GCN layer (reference: PyG GCNConv with self-loops):
    out = Dinv (A + I) Dinv (h) W + b        with Dinv = diag(deg^-1/2)
We fold the two Dinv factors into per-node scalings:
    hs   = dinv * h                 (source-side, before gather)
    agg  = (A + I) hs               (gather + one-hot matmul scatter-add)
    out  = (dinv * agg) W + b       (dst-side scale, then weight matmul)

Perf structure (2.39 ms vs 5.64 ms baseline; bottleneck is GPSIMD/Q7
SWDGE descriptor generation at ~1.7 us per 640-index gather call):
  - layer-1 gather tables are host-precomputed (dinv*x in bf16) and fed as
    input params: no conv-pre phase, no layer-1 AllGather, ~25 us startup.
  - gather calls are aligned to (dst-block, sub-table) groups; each core's
    trailing pad lanes carry idx=-1 (dropped by the ucode's trailing-
    negative trim) with num_idxs_reg loaded per-core from SBUF via
    batched reg_load, so ring reservation == emitted descriptors. This
    cuts ~20% of the random 256B HBM reads and keeps calls single-packet
    (<= 64 descriptors per engine).
  - stale-lane safety: trimmed lanes leave old SBUF bytes in the gather
    tile (masked by dstloc=200 -> S=0), so every pool buffer is memset to
    zero once at startup - 0 * NaN from uninitialized SBUF would
    otherwise poison the psum accumulation.
  - tile stream is dst-half-major then sub-major: per-block epilogues run
    inside each half's last sub pass, and each AllGather fires mid-layer
    with ~half a layer of gather stream to hide behind; table/shard
    buffers ping-pong between layers so an early AllGather never
    overwrites a table the current layer still reads.
  - the self-loop term enters each block's first psum chain as an
    identity matmul; the dst-side dinv rides the transpose matmul as a
    host-built diagonal (dropped entirely for layer 3: the final
    LayerNorm is row-scale-invariant); the next layer's src-side dinv is
    folded into the ReLU scale (h1 residual stored pre-scaled) and into
    the LayerNorm's rstd; LN computes var = E[h^2]-mu^2 and emits
    (h-mu)*rstd as one dual-op tensor_scalar, straight to bf16 hs_pre.
"""

import contextlib
import ctypes
import os
import sys
import types

import numpy as np
import ml_dtypes

import concourse.bass as bass
import concourse.mybir as mybir
import concourse.tile as tile
from concourse import bacc
from concourse.bass_utils import run_bass_kernel_spmd

# ---------------- problem constants (hardcoded per spec) ----------------
N = 100000
E = 1600000
D = 128
P = 128
NCORE = 8
BPC = 98                 # dst blocks of 128 nodes per core
SH = BPC * P             # 12544 nodes per core shard
NPAD = NCORE * SH        # 100352 padded node count
NSUB = 4                 # sub-tables (int16 index reach)
SUBROWS = NPAD // NSUB   # 25088 rows per sub-table
NIMAX_TILES = 5          # max tiles per dma_gather call (= largest group)
MERGE_CALLS = False      # merged calls need multi-packet descriptors, which
                         # drain slower than the saved per-call fixed cost
PADLOC = 200.0           # dstloc value for padding lanes (> 127)
GP_BUFS = 28             # gather-pool depth
NOTRIM_CALLS = 0         # gather-pool buffers are memset once at startup, so
                         # every call can trim its pad lanes (idx=-1); trimmed
                         # lanes then read zeros (finite), never stale NaN bits
LN_EPS = 1e-5

BF16 = mybir.dt.bfloat16
F32 = mybir.dt.float32
I16 = mybir.dt.int16

EXEC_TIME_NS = None      # set by kernel() when profiling is enabled


# ---------------- axon NTFF profile hook (optional) ----------------
def _install_profile_hook():
    so_path = "/opt/axon/libaxon_pjrt.so"
    if "antenv.axon_hooks" in sys.modules:
        return True
    try:
        lib = ctypes.CDLL(so_path)
        if not hasattr(lib, "axon_start_nrt_profile"):
            return False
        lib.axon_start_nrt_profile.argtypes = [ctypes.POINTER(ctypes.c_int64), ctypes.c_size_t]
        lib.axon_start_nrt_profile.restype = ctypes.c_int64
        lib.axon_stop_nrt_profile.argtypes = [ctypes.c_char_p]
        lib.axon_stop_nrt_profile.restype = ctypes.c_int64

        @contextlib.contextmanager
        def _hook(output_dir, device_ids):
            import jax
            jax.devices()
            if device_ids:
                ids = (ctypes.c_int64 * len(device_ids))(*device_ids)
                rc = lib.axon_start_nrt_profile(ids, len(device_ids))
            else:
                rc = lib.axon_start_nrt_profile(None, 0)
            if rc != 0:
                raise RuntimeError(f"axon_start_nrt_profile rc={rc}")
            try:
                yield
            finally:
                n = lib.axon_stop_nrt_profile(str(output_dir).encode())
                if n < 0:
                    raise RuntimeError(f"axon_stop_nrt_profile rc={n}")

        mod = types.ModuleType("antenv.axon_hooks")
        mod.get_axon_ntff_profile_hook = lambda: _hook
        mod.set_axon_ntff_profile_hook = lambda h: None
        sys.modules["antenv.axon_hooks"] = mod

        import concourse.bass_utils as bu
        bu.upload_artifacts = lambda tmpdir: f"local:{tmpdir}"
        return True
    except Exception:
        return False


# ---------------- host-side graph preprocessing ----------------
def _preprocess(edge_index):
    src = np.asarray(edge_index[0], dtype=np.int64)
    dst = np.asarray(edge_index[1], dtype=np.int64)

    deg = np.bincount(dst, minlength=NPAD).astype(np.float64) + 1.0
    dinv = (1.0 / np.sqrt(deg)).astype(np.float32)  # padding nodes -> 1.0

    # table row of node g: owner rank halves are concatenated into two
    # half-tables (A = first 6272 rows of every rank, B = second half).
    HSH = SH // 2
    r_own = src // SH
    off = src % SH
    half = off // HSH
    lrow = r_own * HSH + (off % HSH)          # row within half-table
    sub = half * 2 + lrow // SUBROWS          # 0..3
    srcloc_all = lrow % SUBROWS
    gblk = dst // P                           # global dst block 0..781
    key = gblk * NSUB + sub
    order = np.argsort(key, kind="stable")
    src_s, dst_s, key_s = src[order], dst[order], key[order]
    srcloc_s = srcloc_all[order]

    NKEY = NCORE * BPC * NSUB                 # 784*4 (incl. empty tail blocks)
    cnt = np.bincount(key_s, minlength=NKEY)
    # per (core, local block, sub) counts; blocks 782/783 are zero
    cnt_cbs = cnt.reshape(NCORE, BPC, NSUB)
    T_u = np.ceil(cnt_cbs / P).astype(np.int64).max(axis=0)  # [BPC, NSUB]

    # tile stream: for dst-block half: for s in 0..3: for bl in half.
    # Half-major ordering closes blocks 0-48 (shard A) at ~50% of the layer
    # so the next layer's AllGather A can fire mid-layer; sub-major within a
    # half keeps table_b unneeded until ~25% into the stream.
    HB = BPC // 2 + BPC % 2                   # 49 blocks in the first half
    blorder = [(s, bl) for half in (range(HB), range(HB, BPC))
               for s in range(NSUB) for bl in half]
    NTILES = int(T_u.sum())

    # column base of each (s, bl) group in the tile stream
    group_base = np.zeros((NSUB, BPC), dtype=np.int64)
    cur = 0
    for s, bl in blorder:
        group_base[s, bl] = cur
        cur += int(T_u[bl, s])
    assert cur == NTILES

    # gather calls: merge same-s stream-adjacent groups per call (they read
    # the same sub-table), amortizing the ~1.5us Q7 per-call fixed cost.
    # Only the call-trailing pads (the last group's tail) can be dropped by
    # the ucode's trailing-negative trim; earlier merged groups' pads stay
    # idx=0 and are fetched (cheap, the drain has slack).
    # calls: list of (s, groups, ntiles) with groups = [(bl, T), ...].
    glist = [(s, bl, int(T_u[bl, s])) for s, bl in blorder if T_u[bl, s] > 0]
    calls = []                                # (s, groups, ntiles, off_tiles)
    i = 0
    while i < len(glist):
        s, bl, T = glist[i]
        if T > NIMAX_TILES:                   # oversized group: chunk it
            done = 0
            while done < T:
                ch = min(NIMAX_TILES, T - done)
                calls.append((s, [(bl, ch)], ch, done))
                done += ch
            i += 1
            continue
        groups = [(bl, T)]
        nt = T
        while (MERGE_CALLS and i + 1 < len(glist) and glist[i + 1][0] == s
               and nt + glist[i + 1][2] <= NIMAX_TILES):
            i += 1
            groups.append((glist[i][1], glist[i][2]))
            nt += glist[i][2]
        calls.append((s, groups, nt, 0))
        i += 1

    # tile meta: (bl, s, first_of_group, last_of_group) in stream order
    tilemeta = []
    for s, bl in blorder:
        T = int(T_u[bl, s])
        for t in range(T):
            tilemeta.append((bl, s, t == 0, t == T - 1))

    # per-core edge placement
    starts = np.zeros(NKEY + 1, dtype=np.int64)
    starts[1:] = np.cumsum(cnt)
    rank = np.arange(len(src_s)) - np.repeat(starts[:-1], cnt)

    core_e = gblk[order] // BPC               # owning core of each (sorted) edge
    bl_e = gblk[order] % BPC
    sub_e = key_s % NSUB
    pos = group_base[sub_e, bl_e] * P + rank  # slot in the core's edge stream

    src_local = srcloc_s.astype(np.int16)
    dst_local = (dst_s - (core_e * SH + bl_e * P)).astype(np.float32)

    srcbuf = np.full((NCORE, NTILES * P), -1, dtype=np.int16)
    dstbuf = np.full((NCORE, NTILES * P), PADLOC, dtype=np.float32)
    for c in range(NCORE):
        m = core_e == c
        srcbuf[c, pos[m]] = src_local[m]
        dstbuf[c, pos[m]] = dst_local[m]

    # dstloc sbuf layout: [p, tile]
    dstloc = dstbuf.reshape(NCORE, NTILES, P).transpose(0, 2, 1)  # [c, 128, NTILES]

    # per-core per-call valid-index counts: the gather's num_idxs_reg must
    # equal the number of non-negative indices (the ucode's ring-space
    # reservation and trigger counts are reg-based while descriptor emission
    # is trimmed-data-based; they must agree or stale descriptors fire).
    # Within a call, all groups but the last count full T*128 lanes (their
    # pads are idx=0); the last group is trimmed to this core's edge count.
    ccnt = np.zeros((NCORE, len(calls)), dtype=np.uint32)
    tc0 = 0
    for ci, (s, groups, nt, off) in enumerate(calls):
        if ci < NOTRIM_CALLS:
            ccnt[:, ci] = nt * P
        else:
            bl_last, T_last = groups[-1]
            full = (nt - T_last) * P
            ccnt[:, ci] = full + np.clip(
                cnt_cbs[:, bl_last, s] - off * P, 0, T_last * P)
        tc0 += nt
    assert tc0 == NTILES
    # exact per-call num_idxs (shared immediate): the Q7 widen loop runs
    # ceil(num_idxs/16) iterations, so shave it to the max core's count
    nidx = ((ccnt.max(axis=0).astype(np.int64) + 15) // 16 * 16)

    # pad-lane idx values: -1 (trimmed) only in each call's LAST group;
    # earlier merged groups' pads become 0 (fetched, masked by dstloc=200).
    tc0 = 0
    for ci, (s, groups, nt, off) in enumerate(calls):
        bl_last, T_last = groups[-1]
        lo = tc0 * P
        hi_nontrim = (tc0 + nt - T_last) * P
        if ci < NOTRIM_CALLS:
            hi_nontrim = (tc0 + nt) * P
        seg = srcbuf[:, lo:hi_nontrim]
        seg[seg < 0] = 0
        tc0 += nt
    assert tc0 == NTILES

    # idx16 layout per call: element i -> [i%16, base + i//16], replicated x8.
    idxcols = NTILES * (P // 16)
    idxbuf = np.zeros((NCORE, 16, idxcols), dtype=np.int16)
    tc = 0
    colc = 0
    for ci, (s, groups, nt, off) in enumerate(calls):
        n = nt * P
        blk = srcbuf[:, tc * P:tc * P + n].reshape(NCORE, n // 16, 16)
        idxbuf[:, :, colc:colc + n // 16] = blk.transpose(0, 2, 1)
        tc += nt
        colc += n // 16
    assert tc == NTILES and colc == idxcols
    idx_arr = np.tile(idxbuf, (1, 8, 1))      # [c, 128, idxcols]

    meta = {
        "NTILES": NTILES,
        "IDXCOLS": idxcols,
        "calls": calls,
        "tilemeta": tilemeta,
        "nidx": nidx,
    }
    data = {
        "idx": idx_arr,
        "dstloc": dstloc.astype(ml_dtypes.bfloat16),
        "dinv": dinv,
        "ccnt": ccnt,
    }
    return meta, data


# ---------------- device graph ----------------
def _ap3_iota(iota_t, nt):
    """iota [128,128] viewed as [128, nt, 128] (broadcast middle dim)."""
    a = iota_t[:, :]
    return bass.AP(a.tensor, a.offset, [a.ap[0], [0, nt], a.ap[1]])


def _build_nc(meta, flags):
    NTILES = meta["NTILES"]
    IDXCOLS = meta["IDXCOLS"]
    calls = meta["calls"]
    tilemeta = meta["tilemeta"]
    ln_triv = flags["ln_trivial"]
    fn_triv = flags["fn_trivial"]
    bias_triv = flags["bias_trivial"]
    # the dst dinv may be dropped before the final LN (row-scale-invariant,
    # affine-after-LN unaffected) -- but only when no bias is added between
    # the scale and the LN
    fn_triv_scale = bias_triv[2]

    # 32KB descriptor carveout: deeper per-queue rings keep more gather
    # calls in flight (a 5-tile call reserves ~41 descriptors per ring)
    nc = bacc.Bacc(num_swdge_queues=4, dynamic_dma_scratch_size=32768)

    NCALLS = len(calls)
    idx = nc.declare_dram_parameter("idx", [P, IDXCOLS], I16, isOutput=False)
    dstloc = nc.declare_dram_parameter("dstloc", [P, NTILES], BF16, isOutput=False)
    ccnt_in = nc.declare_dram_parameter("ccnt", [1, NCALLS], mybir.dt.uint32, isOutput=False)
    dinv_in = nc.declare_dram_parameter("dinv", [P, BPC], F32, isOutput=False)
    iota_in = nc.declare_dram_parameter("iota", [P, P], BF16, isOutput=False)
    ident_in = nc.declare_dram_parameter("ident", [P, P], F32, isOutput=False)
    ddiag_in = nc.declare_dram_parameter("ddiag", [SH, P], BF16, isOutput=False)
    identb_in = nc.declare_dram_parameter("identb", [P, P], BF16, isOutput=False)
    # layer-1 gather tables: host-precomputed dinv*x (bf16), replicated;
    # t1sa/t1sb are this core's own shard halves (for the hs_pre load).
    t1a_in = nc.declare_dram_parameter("t1a", [NPAD // 2, D], BF16, isOutput=False)
    t1b_in = nc.declare_dram_parameter("t1b", [NPAD // 2, D], BF16, isOutput=False)
    HSH_ = SH // 2
    t1sa_in = nc.declare_dram_parameter("t1sa", [HSH_, D], BF16, isOutput=False)
    t1sb_in = nc.declare_dram_parameter("t1sb", [HSH_, D], BF16, isOutput=False)
    w_in = [nc.declare_dram_parameter(f"W{k}", [D, D], BF16, isOutput=False) for k in range(3)]
    brep_in = [nc.declare_dram_parameter(f"brep{k}", [P, D], F32, isOutput=False) for k in range(3)]
    lng_in = nc.declare_dram_parameter("lng", [P, D], F32, isOutput=False)
    lnb_in = nc.declare_dram_parameter("lnb", [P, D], F32, isOutput=False)
    fng_in = nc.declare_dram_parameter("fng", [P, D], F32, isOutput=False)
    fnb_in = nc.declare_dram_parameter("fnb", [P, D], F32, isOutput=False)
    out = nc.declare_dram_parameter("out", [SH, D], F32, isOutput=True)

    # ping-pong shard/table buffers: layer 2 uses set 0, layer 3 set 1, so
    # the AllGather for layer k+1 (fired mid-layer-k) never overwrites a
    # table that layer k's remaining gathers still read.
    HSH = SH // 2
    hs_shard = [[nc.dram_tensor(f"hs_shard_{pp_}{h}", [HSH, D], BF16)
                 for h in "ab"] for pp_ in range(2)]
    hs_table = [[nc.dram_tensor(f"hs_table_{pp_}{h}", [NPAD // 2, D], BF16,
                                addr_space="Shared") for h in "ab"] for pp_ in range(2)]
    h1_dram = nc.dram_tensor("h1_dram", [SH, D], F32)

    with tile.TileContext(nc, num_cores=NCORE) as tc:
        with tc.tile_pool(name="persist", bufs=1) as pp, \
             tc.tile_pool(name="stream", bufs=12) as sp, \
             tc.tile_pool(name="gath", bufs=GP_BUFS) as gp, \
             tc.tile_pool(name="epi", bufs=5) as ep, \
             tc.tile_pool(name="dload", bufs=8) as dd, \
             tc.tile_pool(name="psum_agg", bufs=4, space="PSUM") as pa, \
             tc.tile_pool(name="psum_epi", bufs=2, space="PSUM") as pe:

            # ---- persistent loads ----
            from concourse import library_config
            nc.gpsimd.load_library(library_config.mlp)
            idx_sb = pp.tile([P, IDXCOLS], I16)
            c0 = IDXCOLS // 8
            nc.sync.dma_start(idx_sb[:, :c0], idx[:, :c0])
            nc.sync.dma_start(idx_sb[:, c0:], idx[:, c0:])
            dstloc_sb = pp.tile([P, NTILES], BF16)
            nc.sync.dma_start(dstloc_sb[:], dstloc[:])
            ccnt_sb = pp.tile([1, NCALLS], mybir.dt.uint32)
            nc.sync.dma_start(ccnt_sb[:], ccnt_in[:])
            gregs = [nc.gpsimd.alloc_register(f"gather_cnt{i}") for i in range(16)]
            dinv_sb = pp.tile([P, BPC], F32)
            nc.sync.dma_start(dinv_sb[:], dinv_in[:])
            iota_sb = pp.tile([P, P], BF16)
            nc.sync.dma_start(iota_sb[:], iota_in[:])
            ident_sb = pp.tile([P, P], F32)
            nc.sync.dma_start(ident_sb[:], ident_in[:])
            identb_sb = pp.tile([P, P], BF16)
            nc.sync.dma_start(identb_sb[:], identb_in[:])
            w_sb = []
            brep_sb = []
            for k in range(3):
                w = pp.tile([P, D], BF16, name=f"w{k}")
                nc.sync.dma_start(w[:], w_in[k][:])
                w_sb.append(w)
                b = pp.tile([P, D], F32, name=f"brep{k}")
                nc.sync.dma_start(b[:], brep_in[k][:])
                brep_sb.append(b)
            lng_sb = pp.tile([P, D], F32)
            nc.sync.dma_start(lng_sb[:], lng_in[:])
            lnb_sb = pp.tile([P, D], F32)
            nc.sync.dma_start(lnb_sb[:], lnb_in[:])
            fng_sb = pp.tile([P, D], F32)
            nc.sync.dma_start(fng_sb[:], fng_in[:])
            fnb_sb = pp.tile([P, D], F32)
            nc.sync.dma_start(fnb_sb[:], fnb_in[:])
            eps_sb = pp.tile([P, 1], F32)
            nc.vector.memset(eps_sb[:], LN_EPS)

            hs_pre = pp.tile([P, BPC * P], BF16)   # next-gather source, node-major chunks
            acc = pp.tile([P, BPC * P], BF16)      # aggregation accumulators

            # load hs_pre for layer 1 from this core's precomputed shard
            # halves (t1sa/t1sb), reshaping [bl*128+p, d] -> [p, bl*128+d].
            HB = BPC // 2  # 49 blocks per half
            for half, t1s in ((0, t1sa_in), (1, t1sb_in)):
                a = t1s[:, :]
                src_ap = bass.AP(a.tensor, a.offset, [[D, P], [P * D, HB], [1, D]])
                dst = hs_pre[:, half * HB * P:(half + 1) * HB * P]
                dst_ap = bass.AP(dst.tensor, dst.offset, [dst.ap[0], [P, HB], [1, P]])
                nc.sync.dma_start(dst_ap, src_ap)

            def store_chunk(k, bl):
                blk = slice(bl * P, (bl + 1) * P)
                shards = hs_shard[k % 2]
                if bl < BPC // 2:
                    dst = shards[0][bl * P:(bl + 1) * P, :]
                else:
                    dst = shards[1][(bl - BPC // 2) * P:(bl - BPC // 2 + 1) * P, :]
                nc.sync.dma_start(dst, hs_pre[:, blk])

            def emit_ag(k, which):
                shard, table = hs_shard[k % 2][which], hs_table[k % 2][which]
                nc.gpsimd.collective_compute(
                    "AllGather", mybir.AluOpType.bypass,
                    replica_groups=[list(range(NCORE))],
                    ins=[shard[:].opt()], outs=[table[:].opt()],
                )

            def ln_chunk(h, hsum, g_rep, b_rep, trivial, post_scale=None, out_ap=None):
                """LayerNorm of [128,128] f32 chunk -> new tile (f32).

                hsum ([P,1]) is sum(h) from the producer's accum_out, so no
                DVE reduce is needed; sum(h^2) rides a Square activation's
                accum_out and rstd = Rsqrt(ssq/D + (eps - mu^2)) is one ACT
                op. The final (h-mu)*rstd is one dual-op tensor_scalar;
                post_scale ([P,1] AP) folds an extra per-row factor in."""
                mu = ep.tile([P, 1], F32, tag="mu")
                nc.vector.tensor_scalar_mul(mu[:], hsum[:], 1.0 / D)
                sq = ep.tile([P, D], F32, tag="sq")
                ssq = ep.tile([P, 1], F32, tag="ssq")
                nc.scalar.activation(sq[:], h[:], mybir.ActivationFunctionType.Square,
                                     accum_out=ssq[:])
                mu2 = ep.tile([P, 1], F32, tag="mu2")
                nc.scalar.square(mu2[:], mu[:])
                bias = ep.tile([P, 1], F32, tag="lnbias")  # eps - mu^2
                nc.vector.tensor_scalar(bias[:], mu2[:], -1.0, LN_EPS,
                                        op0=mybir.AluOpType.mult,
                                        op1=mybir.AluOpType.add)
                std = ep.tile([P, 1], F32, tag="std")
                nc.scalar.activation(std[:], ssq[:], mybir.ActivationFunctionType.Sqrt,
                                     bias=bias[:, :1], scale=1.0 / D)
                rstd = ep.tile([P, 1], F32, tag="rstd")
                nc.vector.reciprocal(rstd[:], std[:])
                if post_scale is not None:
                    nc.vector.tensor_mul(rstd[:], rstd[:], post_scale)
                if trivial and out_ap is not None:
                    nc.vector.tensor_scalar(out_ap, h[:], mu[:, :1], rstd[:, :1],
                                            op0=mybir.AluOpType.subtract,
                                            op1=mybir.AluOpType.mult)
                    return None
                norm = ep.tile([P, D], F32, tag="norm")
                nc.vector.tensor_scalar(norm[:], h[:], mu[:, :1], rstd[:, :1],
                                        op0=mybir.AluOpType.subtract,
                                        op1=mybir.AluOpType.mult)
                if not trivial:
                    nc.vector.tensor_mul(norm[:], norm[:], g_rep[:])
                    nc.vector.tensor_add(norm[:], norm[:], b_rep[:])
                return norm

            def epilogue(k, bl):
                """Per-block conv epilogue: dst scale, weight matmul, layer
                tail; stores next layer's hs chunk and re-inits acc[bl]."""
                blk = slice(bl * P, (bl + 1) * P)
                # transpose-and-dst-scale in one matmul: rhs = diag(dinv[blk]).
                # For the last layer the dst scale commutes through W and is
                # cancelled by the final LayerNorm, so a plain transpose does.
                if k == 2 and fn_triv_scale:
                    rhs_t = identb_sb
                else:
                    rhs_t = dd.tile([P, P], BF16, tag="ddiag")
                    nc.sync.dma_start(rhs_t[:], ddiag_in[bl * P:(bl + 1) * P, :])
                aggT_p = pe.tile([P, P], F32, tag="aggT", space="PSUM")
                nc.tensor.matmul(aggT_p[:], lhsT=acc[:, blk], rhs=rhs_t[:],
                                 start=True, stop=True)
                aggT = ep.tile([P, P], BF16, tag="aggTs")
                nc.scalar.copy(aggT[:], aggT_p[:])
                o_p = pe.tile([P, P], F32, tag="op", space="PSUM")
                nc.tensor.matmul(o_p[:], lhsT=aggT[:], rhs=w_sb[k][:], start=True, stop=True)

                if k == 0:
                    # h = dinv * relu(o): the src-side dinv of the NEXT layer
                    # folded into the ReLU's scale (dinv > 0 commutes with
                    # relu); the LN downstream is scale-invariant, so h1 can
                    # be stored pre-scaled for layer 2's residual add.
                    h = ep.tile([P, D], F32, tag="h")
                    hsum = ep.tile([P, 1], F32, tag="hsum")
                    if bias_triv[k]:
                        nc.scalar.activation(h[:], o_p[:],
                                             mybir.ActivationFunctionType.Relu,
                                             scale=dinv_sb[:, bl:bl + 1],
                                             accum_out=hsum[:])
                    else:
                        o_b = ep.tile([P, D], F32, tag="ob")
                        nc.vector.tensor_add(o_b[:], o_p[:], brep_sb[k][:])
                        nc.scalar.activation(h[:], o_b[:],
                                             mybir.ActivationFunctionType.Relu,
                                             scale=dinv_sb[:, bl:bl + 1],
                                             accum_out=hsum[:])
                    nc.sync.dma_start(h1_dram[bl * P:(bl + 1) * P, :], h[:])
                    if ln_triv:
                        # LN(dinv*x) = LN(x); fold the outer dinv into rstd
                        # and write hs_pre (bf16) straight out of the LN
                        ln_chunk(h, hsum, lng_sb, lnb_sb, True,
                                 post_scale=dinv_sb[:, bl:bl + 1],
                                 out_ap=hs_pre[:, blk])
                    else:
                        ln = ln_chunk(h, hsum, lng_sb, lnb_sb, False)
                        nc.vector.tensor_scalar_mul(hs_pre[:, blk], ln[:],
                                                    dinv_sb[:, bl:bl + 1])
                    store_chunk(k, bl)
                elif k == 1:
                    h = ep.tile([P, D], F32, tag="h")
                    if bias_triv[k]:
                        nc.scalar.activation(h[:], o_p[:],
                                             mybir.ActivationFunctionType.Relu,
                                             scale=dinv_sb[:, bl:bl + 1])
                    else:
                        o_b = ep.tile([P, D], F32, tag="ob")
                        nc.vector.tensor_add(o_b[:], o_p[:], brep_sb[k][:])
                        nc.scalar.activation(h[:], o_b[:],
                                             mybir.ActivationFunctionType.Relu,
                                             scale=dinv_sb[:, bl:bl + 1])
                    h1c = dd.tile([P, D], F32, tag="h1c")
                    nc.sync.dma_start(h1c[:], h1_dram[bl * P:(bl + 1) * P, :])
                    # hs_pre = dinv*(relu(o)+h1) with both terms pre-scaled
                    nc.vector.tensor_tensor(out=hs_pre[:, blk], in0=h[:], in1=h1c[:],
                                            op=mybir.AluOpType.add)
                    store_chunk(k, bl)
                else:
                    o_b = ep.tile([P, D], F32, tag="ob")
                    hsum = ep.tile([P, 1], F32, tag="hsum")
                    if bias_triv[k]:
                        nc.scalar.activation(o_b[:], o_p[:],
                                             mybir.ActivationFunctionType.Copy,
                                             accum_out=hsum[:])
                    else:
                        nc.vector.tensor_add(o_b[:], o_p[:], brep_sb[k][:])
                        nc.scalar.activation(o_b[:], o_b[:],
                                             mybir.ActivationFunctionType.Copy,
                                             accum_out=hsum[:])
                    ln = ln_chunk(o_b, hsum, fng_sb, fnb_sb, fn_triv)
                    nc.sync.dma_start(out[bl * P:(bl + 1) * P, :], ln[:])

            # zero-seed every gather-pool buffer once: trimmed calls leave
            # unfetched lanes holding whatever the buffer had, and 0*NaN
            # from uninitialized SBUF would poison the psum accumulation
            for _ in range(GP_BUFS):
                gseed = gp.tile([P, NIMAX_TILES, D], BF16, tag="g")
                nc.vector.memset(gseed[:], 0.0)

            for k in range(3):  # conv layers
                if k == 0:
                    tabs = (t1a_in, t1b_in)
                else:
                    # layer k reads the set written by layer k-1's epilogue
                    tabs = tuple(hs_table[(k - 1) % 2])

                tcur = 0          # tile cursor
                ccur = 0          # idx col cursor
                qrr = 0
                cur_psum = None
                cur_bl = None
                for ci, (s, groups, nt, off) in enumerate(calls):
                    g = gp.tile([P, NIMAX_TILES, D], BF16, tag="g")
                    tab = tabs[0] if s < 2 else tabs[1]
                    soff = (s % 2) * SUBROWS
                    if ci % 16 == 0:
                        nb = min(16, len(calls) - ci)
                        nc.gpsimd.reg_load(gregs[:nb], ccnt_sb[0:1, ci:ci + nb])
                    n_i = int(meta["nidx"][ci])
                    nc.gpsimd.dma_gather(
                        out_ap=g[:, :nt, :],
                        in_ap=tab[soff:soff + SUBROWS, :],
                        idxs_ap=idx_sb[:, ccur:ccur + (n_i + 15) // 16],
                        num_idxs=n_i, num_idxs_reg=gregs[ci % 16], elem_size=D,
                        # a single packet holds <=64 descriptors per engine
                        # (= 1024 indices); larger merged calls go multi-packet
                        single_packet=(nt * P <= 1024),
                        queue_num=qrr,
                    )
                    qrr = (qrr + 1) % 4
                    S = sp.tile([P, NIMAX_TILES, P], BF16, tag="S")
                    nc.vector.tensor_tensor(
                        out=S[:, :nt, :],
                        in0=dstloc_sb[:, tcur:tcur + nt].to_broadcast([P, nt, P]),
                        in1=_ap3_iota(iota_sb, nt),
                        op=mybir.AluOpType.is_equal)
                    for t in range(nt):
                        bl_, s_, first, last = tilemeta[tcur + t]
                        blk = slice(bl_ * P, (bl_ + 1) * P)
                        if first:
                            cur_psum = pa.tile([P, P], F32, tag="agg", space="PSUM")
                            cur_bl = bl_
                            if s_ == 0:
                                # self-loop term folded into the psum chain
                                nc.tensor.matmul(cur_psum[:], lhsT=identb_sb[:],
                                                 rhs=hs_pre[:, blk],
                                                 start=True, stop=False)
                        assert cur_bl == bl_
                        nc.tensor.matmul(cur_psum[:], lhsT=S[:, t, :], rhs=g[:, t, :],
                                         start=(first and s_ != 0), stop=last)
                        if last:
                            with tc.high_priority(offset=200):
                                if s_ == 0:
                                    nc.vector.tensor_copy(acc[:, blk], cur_psum[:])
                                else:
                                    nc.vector.tensor_add(
                                        acc[:, blk], acc[:, blk], cur_psum[:])
                            if s_ == NSUB - 1:
                                epilogue(k, bl_)
                                if k < 2:
                                    if bl_ == BPC // 2 - 1:
                                        emit_ag(k, 0)
                                    elif bl_ == BPC - 1:
                                        emit_ag(k, 1)
                    tcur += nt
                    ccur += nt * (P // 16)
                assert tcur == NTILES

    nc.finalize()
    return nc


# ---------------- entry point ----------------
def kernel(x, edge_index, W0, b0, W1, b1, W2, b2, ln_g, ln_b, fn_g, fn_b):
    global EXEC_TIME_NS
    x = np.asarray(x, dtype=np.float32)
    meta, data = _preprocess(edge_index)

    flags = {
        "ln_trivial": bool(np.all(np.asarray(ln_g) == 1.0) and np.all(np.asarray(ln_b) == 0.0)),
        "fn_trivial": bool(np.all(np.asarray(fn_g) == 1.0) and np.all(np.asarray(fn_b) == 0.0)),
        "bias_trivial": [bool(np.all(np.asarray(b) == 0.0)) for b in (b0, b1, b2)],
    }
    nc = _build_nc(meta, flags)

    # host-precompute the layer-1 gather table: dinv * x, padded, bf16,
    # in the half-table layout (A = first 6272 rows of every rank's shard).
    HSH = SH // 2
    hs1 = np.zeros((NPAD, D), dtype=np.float32)
    hs1[:N] = x * data["dinv"][:N, None]
    hs1 = hs1.astype(ml_dtypes.bfloat16)
    shards = hs1.reshape(NCORE, SH, D)
    t1a = np.ascontiguousarray(shards[:, :HSH, :].reshape(NPAD // 2, D))
    t1b = np.ascontiguousarray(shards[:, HSH:, :].reshape(NPAD // 2, D))

    iota_arr = np.tile(np.arange(P, dtype=np.float32)[None, :], (P, 1)).astype(ml_dtypes.bfloat16)
    ident_arr = np.eye(P, dtype=np.float32)

    def rep(v):
        return np.tile(np.asarray(v, np.float32)[None, :], (P, 1))

    in_maps = []
    for c in range(NCORE):
        dinv_c = data["dinv"][c * SH:(c + 1) * SH].reshape(BPC, P).T.copy()  # [p, bl]
        ddiag_c = np.zeros((SH, P), dtype=ml_dtypes.bfloat16)
        rows = np.arange(SH)
        ddiag_c[rows, rows % P] = data["dinv"][c * SH:(c + 1) * SH]
        in_maps.append({
            "idx": data["idx"][c],
            "dstloc": data["dstloc"][c],
            "dinv": np.ascontiguousarray(dinv_c),
            "ddiag": ddiag_c,
            "ccnt": data["ccnt"][c][None, :],
            "iota": iota_arr,
            "ident": ident_arr,
            "identb": ident_arr.astype(ml_dtypes.bfloat16),
            "t1a": t1a, "t1b": t1b,
            "t1sa": np.ascontiguousarray(t1a[c * HSH:(c + 1) * HSH]),
            "t1sb": np.ascontiguousarray(t1b[c * HSH:(c + 1) * HSH]),
            "W0": np.asarray(W0, ml_dtypes.bfloat16), "W1": np.asarray(W1, ml_dtypes.bfloat16),
            "W2": np.asarray(W2, ml_dtypes.bfloat16),
            "brep0": rep(b0), "brep1": rep(b1), "brep2": rep(b2),
            "lng": rep(ln_g), "lnb": rep(ln_b),
            "fng": rep(fn_g), "fnb": rep(fn_b),
        })

    profile = bool(os.environ.get("GNN_PROFILE")) and _install_profile_hook()
    res = run_bass_kernel_spmd(nc, in_maps, core_ids=list(range(NCORE)), trace=profile)
    EXEC_TIME_NS = res.exec_time_ns

    out = np.concatenate([res.results[c]["out"] for c in range(NCORE)], axis=0)
    return out[:N]
